# revision 1
# baseline (speedup 1.0000x reference)
"""MoE (BruteForceMoELinear) Trainium2 kernel — bf16 expert-parallel.

Strategy: expert-parallel across 8 NeuronCores.  The host dispatches
token rows by `gate_idx` (stable sort), folds the per-row gate score
into the activations (scores >= 0 commute through ReLU), pads each
expert's batch to capacity C, and hands core e bf16-packed operands.

Per-core compute: y_e^T = W2_e @ relu(W1_e @ x_e^T), bf16 matmuls with
fp32 PSUM accumulation.  Tokens split into a big chunk A (<=512 cols)
and a small remainder B.  GEMM1 opens ko-major over the first FO1
f-groups so the PE consumes each (W1-ko, x-ko) row-DMA the moment it
lands; W1-ko and x-ko are packed into a single DRAM row per ko to
minimize per-DMA descriptor-generation serialization.  The rest runs
fo-major against streamed W1, with B's tiny groups woven between A
groups.  GEMM2 ends with a column-split last d-group so the final
evict+DMA tail is short.  PSUM evictions alternate between the scalar
and vector engines.
"""

import numpy as np
import ml_dtypes

import os

NUM_EXPERT = 8
N_CORES = 8
P = 128
FO1 = int(os.environ.get("K_FO1", "6"))  # ko-major head fo-groups
_CUT = int(os.environ.get("K_CUT", "2"))     # W1 cols in first DMA piece
_SUBS = os.environ.get("K_SUBS", "12,4")     # last d-group col split /16
_LASTACT = int(os.environ.get("K_LASTACT", "1"))  # 1: evict subs Act-first

_CACHE = {}


def _chunks_for(C):
    if C <= 512:
        return [C]
    assert C <= 1024
    return [512, C - 512]


def _build(C, KO, FO, repeat=1):
    key = (C, KO, FO, repeat)
    if key in _CACHE:
        return _CACHE[key]

    import concourse.mybir as mybir
    import concourse.tile as tile
    from concourse import bacc

    f32 = mybir.dt.float32
    bf16 = mybir.dt.bfloat16
    chunks = _chunks_for(C)
    TA = chunks[0]
    TB = chunks[1] if len(chunks) > 1 else 0
    nfo1 = min(FO1, FO)
    FOB = FO - nfo1
    RS = TA + nfo1 * P           # row stride: x-ko | w1a-ko
    XWN = KO * RS + KO * TB      # + xB appended at the end

    nc = bacc.Bacc("TRN2", target_bir_lowering=False, debug=False,
                   num_devices=N_CORES)

    xw = nc.dram_tensor("xw", (P, XWN), bf16, kind="ExternalInput")
    w1b = nc.dram_tensor("w1b", (P, FOB, KO * P), bf16, kind="ExternalInput")
    w2 = nc.dram_tensor("w2", (P, KO, FO * P), bf16, kind="ExternalInput")
    yt = nc.dram_tensor("yt", (P, KO * C), bf16, kind="ExternalOutput")

    with tile.TileContext(nc) as tc:
        with tc.tile_pool(name="wpool", bufs=1) as wpool, \
             tc.tile_pool(name="ypool", bufs=4) as ypool, \
             tc.tile_pool(name="psA", bufs=6, space="PSUM") as psA, \
             tc.tile_pool(name="psB", bufs=2, space="PSUM") as psB:

            xwsb = wpool.tile([P, XWN], bf16, name="xwsb")
            w1bsb = (wpool.tile([P, FOB, KO * P], bf16, name="w1bsb")
                     if FOB else None)
            w2sb = wpool.tile([P, KO, FO * P], bf16, name="w2sb")
            hA = wpool.tile([P, FO, TA], bf16, name="hA")
            hB = wpool.tile([P, FO, TB], bf16, name="hB") if TB else None

            def xA_ap(ko):
                return xwsb[:, ko * RS:ko * RS + TA]

            def xB_ap(ko):
                return xwsb[:, KO * RS + ko * TB:KO * RS + (ko + 1) * TB]

            def w1_ap(f, ko):
                if f < nfo1:
                    off = ko * RS + TA + f * P
                    return xwsb[:, off:off + P]
                return w1bsb[:, f - nfo1, ko * P:(ko + 1) * P]

            # --- DMAs: emission order == consumption order -------------
            cut = TA + _CUT * P if nfo1 >= _CUT else RS
            nc.sync.dma_start(xwsb[:, 0:cut], xw.ap()[:, 0:cut])
            if cut < RS:
                nc.sync.dma_start(xwsb[:, cut:RS], xw.ap()[:, cut:RS])
            for ko in range(1, KO):
                hi = (ko + 1) * RS if ko < KO - 1 else XWN
                nc.sync.dma_start(xwsb[:, ko * RS:hi], xw.ap()[:, ko * RS:hi])
            fo = 0
            while fo < FOB:
                hi = min(fo + int(os.environ.get("K_W1B", "4")), FOB)
                nc.sync.dma_start(w1bsb[:, fo:hi, :], w1b.ap()[:, fo:hi, :])
                fo = hi
            nc.sync.dma_start(w2sb[:, 0:2, :], w2.ap()[:, 0:2, :])
            nc.sync.dma_start(w2sb[:, 2:KO, :], w2.ap()[:, 2:KO, :])

            def evict1(dst, src, use_act):
                if use_act:
                    nc.scalar.activation(dst, src,
                                         mybir.ActivationFunctionType.Relu)
                else:
                    nc.vector.tensor_scalar_max(dst, src, 0.0)

            # Keep-warm bridge: the cost model resets the PE p-state
            # anchor when the PE idles more than ~0.8us, and the ramp to
            # full rate takes 3us from the anchor.  Emit a chain of tiny
            # matmuls, each gated by a ~0.6us Pool-engine memset, so PE
            # activity recurs every <0.7us until the first real matmul
            # (~3.5us, after the row-0 DMA) — which then runs full-rate.
            warm = wpool.tile([P, 16], bf16, name="warm")
            NPACE = 18
            pace = wpool.tile([P, (NPACE + 1) * 16], bf16, name="pace")
            bconst = nc.const_aps.aps[(mybir.dt.bfloat16, 1.0)]
            nc.tensor.ldweights(bconst)
            nc.vector.memset(warm[:], 0.0)
            nc.vector.memset(pace[:, 0:16], 0.0)
            wps = psB.tile([P, 16], f32, name="wps", tag="pB")
            nc.tensor.matmul(wps[0:16, :], warm[:], warm[:],
                             start=True, stop=True)
            for k in range(NPACE):
                nc.vector.tensor_scalar_add(
                    pace[:, (k + 1) * 16:(k + 2) * 16],
                    pace[:, k * 16:(k + 1) * 16], 0.0)
                nc.tensor.matmul(
                    wps[0:16, :], warm[:],
                    pace[:, (k + 1) * 16:(k + 2) * 16],
                    start=True, stop=True)

            # --- GEMM1 phase 1: ko-major over fo 0..nfo1 on chunk A ----
            p1s = [psA.tile([P, TA], f32, name=f"p1f{f}", tag="pA")
                   for f in range(nfo1)]
            for ko in range(KO):
                for f in range(nfo1):
                    nc.tensor.matmul(p1s[f][:], w1_ap(f, ko), xA_ap(ko),
                                     start=(ko == 0), stop=(ko == KO - 1))
                    if ko == KO - 1:
                        evict1(hA[:, f, :], p1s[f][:], f % 2 == 0)

            # --- GEMM1 phase 2: fo-major, B's groups interleaved -------
            def gemm1B(f):
                pb = psB.tile([P, TB], f32, name="pb", tag="pB")
                for ko in range(KO):
                    nc.tensor.matmul(pb[:], w1_ap(f, ko), xB_ap(ko),
                                     start=(ko == 0), stop=(ko == KO - 1))
                nc.vector.tensor_scalar_max(hB[:, f, :], pb[:], 0.0)

            bq = list(range(FO)) if TB else []
            NB = len(bq)
            nA2 = max(FO - nfo1, 1)
            for i, f in enumerate(range(nfo1, FO)):
                p1 = psA.tile([P, TA], f32, name="p1", tag="pA")
                for ko in range(KO):
                    nc.tensor.matmul(p1[:], w1_ap(f, ko), xA_ap(ko),
                                     start=(ko == 0), stop=(ko == KO - 1))
                evict1(hA[:, f, :], p1[:], True)
                ntake = ((i + 1) * NB) // nA2 - (i * NB) // nA2
                for _ in range(ntake):
                    gemm1B(bq.pop(0))
            for f in bq:
                gemm1B(f)

            # --- GEMM2 --------------------------------------------------
            def gemm2A(do, c0, c1, ysb, use_act=True, dma_eng=None):
                p2 = psA.tile([P, TA], f32, name="p2", tag="pA")
                for f in range(FO):
                    nc.tensor.matmul(p2[:, 0:c1 - c0],
                                     w2sb[:, do, f * P:(f + 1) * P],
                                     hA[:, f, c0:c1],
                                     start=(f == 0), stop=(f == FO - 1))
                if use_act:
                    nc.scalar.copy(ysb[:], p2[:, 0:c1 - c0])
                else:
                    nc.vector.tensor_scalar_add(ysb[:], p2[:, 0:c1 - c0], 0.0)
                (dma_eng or nc.sync).dma_start(
                    yt.ap()[:, do * TA + c0:do * TA + c1], ysb[:])

            def gemm2B(do, ysbB):
                pb = psB.tile([P, TB], f32, name="p2b", tag="pB")
                for f in range(FO):
                    nc.tensor.matmul(pb[:],
                                     w2sb[:, do, f * P:(f + 1) * P],
                                     hB[:, f, :],
                                     start=(f == 0), stop=(f == FO - 1))
                nc.vector.tensor_scalar_add(
                    ysbB[:, do * TB:(do + 1) * TB], pb[:], 0.0)

            ysbB = (ypool.tile([P, KO * TB], bf16, tag="yB", name="yB")
                    if TB else None)
            for do in range(KO - 1):
                ysb = ypool.tile([P, TA], bf16, tag="yA", name="yA")
                gemm2A(do, 0, TA, ysb)
                if TB:
                    gemm2B(do, ysbB)
            if TB:
                gemm2B(KO - 1, ysbB)
                nc.sync.dma_start(yt.ap()[:, KO * TA:KO * C], ysbB[:])

            # last A d-group, column-split with decreasing subgroups so
            # the terminal output DMAs stay spaced >= the HWDGE fixed
            # cost and the final evict+DMA covers few columns.
            if TA >= 512:
                subs = [TA * int(v) // 16 for v in _SUBS.split(",")]
            else:
                subs = [TA]
            c0 = 0
            for s, sub in enumerate(subs):
                ysb = ypool.tile([P, sub], bf16, tag="yA3", name="yA3")
                # non-final subs issue their output DMA from the scalar
                # queue (right behind their own eviction) so the SP queue
                # is free to issue the final sub's DMA the moment its
                # eviction lands, instead of waiting out the previous
                # DMA's descriptor-generation hold.
                gemm2A(KO - 1, c0, c0 + sub, ysb,
                       use_act=(s % 2 == 0) == bool(_LASTACT),
                       dma_eng=nc.scalar if s < len(subs) - 1 else None)
                c0 += sub

    nc.compile()
    _CACHE[key] = nc
    return nc


_last = {}


def _pack_inputs(xs, w_htoh4, w_h4toh, idx_split, C, KO, FO):
    bf16 = ml_dtypes.bfloat16
    chunks = _chunks_for(C)
    TA = chunks[0]
    TB = chunks[1] if len(chunks) > 1 else 0
    nfo1 = min(FO1, FO)
    RS = TA + nfo1 * P
    d_model = KO * P
    in_maps = []
    for e in range(NUM_EXPERT):
        idx = idx_split[e]
        cnt = len(idx)
        xT = np.zeros((d_model, C), dtype=np.float32)
        if cnt:
            xT[:, :cnt] = xs[idx].T
        xk = xT.reshape(KO, P, C)                          # [ko, p, c]
        w1t = w_htoh4[e].T.reshape(KO, P, FO, P)          # [ko, p, fo, f]
        rows = []
        for ko in range(KO):
            rows.append(xk[ko, :, :TA])                   # x-ko  (P, TA)
            rows.append(w1t[ko, :, :nfo1, :].reshape(P, nfo1 * P))
        xw_h = np.concatenate(rows, axis=1)               # (P, KO*RS)
        if TB:
            xB = xk[:, :, TA:C].transpose(1, 0, 2).reshape(P, KO * TB)
            xw_h = np.concatenate([xw_h, xB], axis=1)
        w1b_h = w1t[:, :, nfo1:, :].transpose(1, 2, 0, 3) \
            .reshape(P, FO - nfo1, KO * P)
        w2t = w_h4toh[e].T.reshape(FO, P, KO, P)          # [fo, p, do, d]
        w2_h = w2t.transpose(1, 2, 0, 3).reshape(P, KO, FO * P)
        in_maps.append({
            "xw": np.ascontiguousarray(xw_h.astype(bf16)),
            "w1b": np.ascontiguousarray(w1b_h.astype(bf16)),
            "w2": np.ascontiguousarray(w2_h.astype(bf16)),
        })
    return in_maps


def kernel(inp, gate_idx, gate_score, w_htoh4, w_h4toh):
    inp = np.ascontiguousarray(np.asarray(inp, dtype=np.float32))
    gate_idx = np.asarray(gate_idx)
    gate_score = np.asarray(gate_score, dtype=np.float32)
    w_htoh4 = np.asarray(w_htoh4, dtype=np.float32)
    w_h4toh = np.asarray(w_h4toh, dtype=np.float32)

    B, d_model = inp.shape
    n_expert, d_ff, _ = w_htoh4.shape
    assert n_expert == NUM_EXPERT
    KO = d_model // P
    FO = d_ff // P

    gi = gate_idx.astype(np.int64)
    order = np.argsort(gi, kind="stable")
    counts = np.bincount(gi, minlength=NUM_EXPERT)
    idx_split = np.split(order, np.cumsum(counts)[:-1])

    C = max(int(-(-counts.max() // 16) * 16), 256)
    TA = _chunks_for(C)[0]

    scores_flat = gate_score.reshape(-1)
    xs = inp * scores_flat[:, None]

    nc = _build(C, KO, FO)
    in_maps = _pack_inputs(xs, w_htoh4, w_h4toh, idx_split, C, KO, FO)

    from concourse import bass_utils
    res = bass_utils.run_bass_kernel_spmd(nc, in_maps,
                                          core_ids=list(range(N_CORES)))

    _last.update(nc=nc, in_maps=in_maps, res=res, C=C, KO=KO, FO=FO)

    y_full = np.empty((B, d_model), dtype=np.float32)
    for e in range(NUM_EXPERT):
        idx = idx_split[e]
        if len(idx) == 0:
            continue
        yt_h = res.results[e]["yt"].astype(np.float32)  # (P, KO*C)
        yA = yt_h[:, :KO * TA].reshape(P, KO, TA)
        if C > TA:
            yB = yt_h[:, KO * TA:].reshape(P, KO, C - TA)
            yk = np.concatenate([yA, yB], axis=2)
        else:
            yk = yA
        yT = yk.transpose(1, 0, 2).reshape(d_model, C)
        y_full[idx] = yT[:, :len(idx)].T
    out = y_full[0::2] + y_full[1::2]
    return np.ascontiguousarray(out, dtype=np.float32)



# revision 11
# speedup vs baseline: 1.0143x; 1.0143x over previous
"""MoE (BruteForceMoELinear) Trainium2 kernel — bf16 expert-parallel.

Strategy: expert-parallel across 8 NeuronCores.  The host dispatches
token rows by `gate_idx` (stable sort), folds the per-row gate score
into the activations (scores >= 0 commute through ReLU), pads each
expert's batch to capacity C, and hands core e bf16-packed operands.

Per-core compute: y_e^T = W2_e @ relu(W1_e @ x_e^T), bf16 matmuls with
fp32 PSUM accumulation.  Tokens split into a big chunk A (<=512 cols)
and a small remainder B.  GEMM1 opens ko-major over the first FO1
f-groups so the PE consumes each (W1-ko, x-ko) row-DMA the moment it
lands; W1-ko and x-ko are packed into a single DRAM row per ko to
minimize per-DMA descriptor-generation serialization.  The rest runs
fo-major against streamed W1, with B's tiny groups woven between A
groups.

Output tail: a plain HWDGE store pays ~625ns descriptor-gen + ~650ns
DGE->DMA latency + ~900ns completion-semaphore propagation after the
final eviction, so the last d-group is drained through SWDGE instead —
dma_scatter_add descriptors are pre-generated on the Pool engine
mid-kernel (prepare_only) into per-piece queues, and each piece fires
with a cheap trigger_dma the moment its PSUM eviction lands (dest
pre-zeroed, so add == store).  The small B remainder and the earlier
d-groups finish long before the end and keep the normal HWDGE path.
"""

import numpy as np
import ml_dtypes

import os

NUM_EXPERT = 8
N_CORES = 8
P = 128
FO1 = int(os.environ.get("K_FO1", "6"))  # ko-major head fo-groups
_CUT = int(os.environ.get("K_CUT", "2"))     # W1 cols in first DMA piece
_S1 = int(os.environ.get("K_S1", "128"))     # final scatter piece cols

_CACHE = {}


def _chunks_for(C):
    if C <= 512:
        return [C]
    assert C <= 1024
    return [512, C - 512]


def _build(C, KO, FO, repeat=1):
    key = (C, KO, FO, repeat)
    if key in _CACHE:
        return _CACHE[key]

    import concourse.mybir as mybir
    import concourse.tile as tile
    from concourse import bacc
    from concourse.instruction_name_ordered_set import InstructionNameOrderedSet

    def _nsdep(inst, dep):
        s = InstructionNameOrderedSet()
        s.add(dep.ins.name)
        inst.ins.add_nosync_dependencies_from(s)

    f32 = mybir.dt.float32
    bf16 = mybir.dt.bfloat16
    i16 = mybir.dt.int16
    chunks = _chunks_for(C)
    TA = chunks[0]
    TB = chunks[1] if len(chunks) > 1 else 0
    nfo1 = min(FO1, FO)
    FOB = FO - nfo1
    RS = TA + nfo1 * P           # row stride: x-ko | w1a-ko
    XWN = KO * RS + KO * TB      # + xB appended at the end
    use_sw = (TA % 128 == 0) and TA >= 2 * _S1  # SWDGE tail path
    S1 = _S1
    S0 = TA - S1

    nc = bacc.Bacc("TRN2", target_bir_lowering=False, debug=False,
                   num_devices=N_CORES,
                   num_swdge_queues=2 if use_sw else 1)

    xw = nc.dram_tensor("xw", (P, XWN), bf16, kind="ExternalInput")
    w1b = nc.dram_tensor("w1b", (P, FOB, KO * P), bf16, kind="ExternalInput")
    w2 = nc.dram_tensor("w2", (P, KO, FO * P), bf16, kind="ExternalInput")
    yt = nc.dram_tensor("yt", (P, KO * C), bf16, kind="ExternalOutput")
    if use_sw:
        sidx = nc.dram_tensor("sidx", (P, 8), i16, kind="ExternalInput")
        ylast = nc.dram_tensor("ylast", (P, TA), f32, kind="ExternalOutput")

    with tile.TileContext(nc) as tc:
        with tc.tile_pool(name="wpool", bufs=1) as wpool, \
             tc.tile_pool(name="ypool", bufs=4) as ypool, \
             tc.tile_pool(name="psA", bufs=6, space="PSUM") as psA, \
             tc.tile_pool(name="psB", bufs=2, space="PSUM") as psB:

            xwsb = wpool.tile([P, XWN], bf16, name="xwsb")
            w1bsb = (wpool.tile([P, FOB, KO * P], bf16, name="w1bsb")
                     if FOB else None)
            w2sb = wpool.tile([P, KO, FO * P], bf16, name="w2sb")
            hA = wpool.tile([P, FO, TA], bf16, name="hA")
            hB = wpool.tile([P, FO, TB], bf16, name="hB") if TB else None
            if use_sw:
                sidxsb = wpool.tile([P, 8], i16, name="sidxsb")
                ysl0 = wpool.tile([P, 1, S0], f32, name="ysl0")
                ysl1 = wpool.tile([P, 1, S1], f32, name="ysl1")
                guard = wpool.tile([P, 16], f32, name="guard")

            def xA_ap(ko):
                return xwsb[:, ko * RS:ko * RS + TA]

            def xB_ap(ko):
                return xwsb[:, KO * RS + ko * TB:KO * RS + (ko + 1) * TB]

            def w1_ap(f, ko):
                if f < nfo1:
                    off = ko * RS + TA + f * P
                    return xwsb[:, off:off + P]
                return w1bsb[:, f - nfo1, ko * P:(ko + 1) * P]

            # --- DMAs: emission order == consumption order -------------
            cut = TA + _CUT * P if nfo1 >= _CUT else RS
            nc.sync.dma_start(xwsb[:, 0:cut], xw.ap()[:, 0:cut])
            if cut < RS:
                nc.sync.dma_start(xwsb[:, cut:RS], xw.ap()[:, cut:RS])
            for ko in range(1, KO):
                hi = (ko + 1) * RS if ko < KO - 1 else XWN
                nc.sync.dma_start(xwsb[:, ko * RS:hi], xw.ap()[:, ko * RS:hi])
            fo = 0
            while fo < FOB:
                hi = min(fo + int(os.environ.get("K_W1B", "4")), FOB)
                nc.sync.dma_start(w1bsb[:, fo:hi, :], w1b.ap()[:, fo:hi, :])
                fo = hi
            if use_sw:
                nc.sync.dma_start(sidxsb[:], sidx.ap()[:])
            nc.sync.dma_start(w2sb[:, 0:2, :], w2.ap()[:, 0:2, :])
            nc.sync.dma_start(w2sb[:, 2:KO, :], w2.ap()[:, 2:KO, :])
            if use_sw:
                # No explicit zeroing of ylast: both execution paths
                # (native run_neff and the PJRT redirect) pre-zero
                # ExternalOutput buffers, so scatter-add == store.
                sem0 = nc.alloc_semaphore("sdma0")
                sem1 = nc.alloc_semaphore("sdma1")
                nc.gpsimd.dma_scatter_add(
                    ylast.ap()[:, 0:S0], ysl0[:], sidxsb[:], P, P, S0,
                    elem_step=TA, prepare_only=True, sem=sem0, queue_num=0,
                    single_packet=False)
                nc.gpsimd.dma_scatter_add(
                    ylast.ap()[:, S0:TA], ysl1[:], sidxsb[:], P, P, S1,
                    elem_step=TA, prepare_only=True, sem=sem1, queue_num=1,
                    single_packet=False)

            def evict1(dst, src, use_act):
                if use_act:
                    nc.scalar.activation(dst, src,
                                         mybir.ActivationFunctionType.Relu)
                else:
                    nc.vector.tensor_scalar_max(dst, src, 0.0)

            # Keep-warm bridge: the cost model resets the PE p-state
            # anchor when the PE idles more than ~0.8us, and the ramp to
            # full rate takes 3us from the anchor.  Emit a chain of tiny
            # matmuls, each gated by a ~0.6us Pool-engine memset, so PE
            # activity recurs every <0.7us until the first real matmul
            # (~3.5us, after the row-0 DMA) — which then runs full-rate.
            warm = wpool.tile([P, 16], bf16, name="warm")
            NPACE = 18
            pace = wpool.tile([P, (NPACE + 1) * 16], bf16, name="pace")
            bconst = nc.const_aps.aps[(mybir.dt.bfloat16, 1.0)]
            nc.tensor.ldweights(bconst)
            nc.vector.memset(warm[:], 0.0)
            nc.vector.memset(pace[:, 0:16], 0.0)
            wps = psB.tile([P, 16], f32, name="wps", tag="pB")
            nc.tensor.matmul(wps[0:16, :], warm[:], warm[:],
                             start=True, stop=True)
            for k in range(NPACE):
                nc.vector.tensor_scalar_add(
                    pace[:, (k + 1) * 16:(k + 2) * 16],
                    pace[:, k * 16:(k + 1) * 16], 0.0)
                nc.tensor.matmul(
                    wps[0:16, :], warm[:],
                    pace[:, (k + 1) * 16:(k + 2) * 16],
                    start=True, stop=True)

            # --- GEMM1 phase 1: ko-major over fo 0..nfo1 on chunk A ----
            p1s = [psA.tile([P, TA], f32, name=f"p1f{f}", tag="pA")
                   for f in range(nfo1)]
            for ko in range(KO):
                for f in range(nfo1):
                    nc.tensor.matmul(p1s[f][:], w1_ap(f, ko), xA_ap(ko),
                                     start=(ko == 0), stop=(ko == KO - 1))
                    if ko == KO - 1:
                        evict1(hA[:, f, :], p1s[f][:], f % 2 == 0)

            # --- GEMM1 phase 2: fo-major, B's groups interleaved -------
            def gemm1B(f):
                pb = psB.tile([P, TB], f32, name="pb", tag="pB")
                for ko in range(KO):
                    nc.tensor.matmul(pb[:], w1_ap(f, ko), xB_ap(ko),
                                     start=(ko == 0), stop=(ko == KO - 1))
                nc.vector.tensor_scalar_max(hB[:, f, :], pb[:], 0.0)

            bq = list(range(FO)) if TB else []
            NB = len(bq)
            nA2 = max(FO - nfo1, 1)
            for i, f in enumerate(range(nfo1, FO)):
                p1 = psA.tile([P, TA], f32, name="p1", tag="pA")
                for ko in range(KO):
                    nc.tensor.matmul(p1[:], w1_ap(f, ko), xA_ap(ko),
                                     start=(ko == 0), stop=(ko == KO - 1))
                evict1(hA[:, f, :], p1[:], True)
                ntake = ((i + 1) * NB) // nA2 - (i * NB) // nA2
                for _ in range(ntake):
                    gemm1B(bq.pop(0))
            for f in bq:
                gemm1B(f)

            # --- GEMM2 --------------------------------------------------
            def gemm2A(do, c0, c1, ysb, use_act=True, dma_eng=None):
                p2 = psA.tile([P, TA], f32, name="p2", tag="pA")
                for f in range(FO):
                    nc.tensor.matmul(p2[:, 0:c1 - c0],
                                     w2sb[:, do, f * P:(f + 1) * P],
                                     hA[:, f, c0:c1],
                                     start=(f == 0), stop=(f == FO - 1))
                if use_act:
                    nc.scalar.copy(ysb[:], p2[:, 0:c1 - c0])
                else:
                    nc.vector.tensor_scalar_add(ysb[:], p2[:, 0:c1 - c0], 0.0)
                (dma_eng or nc.sync).dma_start(
                    yt.ap()[:, do * TA + c0:do * TA + c1], ysb[:])

            def gemm2B(do, ysbB):
                pb = psB.tile([P, TB], f32, name="p2b", tag="pB")
                for f in range(FO):
                    nc.tensor.matmul(pb[:],
                                     w2sb[:, do, f * P:(f + 1) * P],
                                     hB[:, f, :],
                                     start=(f == 0), stop=(f == FO - 1))
                nc.vector.tensor_scalar_add(
                    ysbB[:, do * TB:(do + 1) * TB], pb[:], 0.0)

            # B first: its tokens finish (and store) long before the
            # tail so its HWDGE latency chain never gates the kernel end.
            if TB:
                ysbB = ypool.tile([P, KO * TB], bf16, tag="yB", name="yB")
                for do in range(KO):
                    gemm2B(do, ysbB)
                nc.sync.dma_start(yt.ap()[:, KO * TA:KO * C], ysbB[:])
            for do in range(KO - 1):
                ysb = ypool.tile([P, TA], bf16, tag="yA", name="yA")
                gemm2A(do, 0, TA, ysb)

            if use_sw:
                # Last d-group via pre-generated SWDGE descriptors: the
                # trigger skips HWDGE desc-gen and the DGE->DMA handoff,
                # so the final store starts right after its eviction.
                p2 = psA.tile([P, TA], f32, name="p2s0", tag="pA")
                for f in range(FO):
                    nc.tensor.matmul(p2[:, 0:S0],
                                     w2sb[:, KO - 1, f * P:(f + 1) * P],
                                     hA[:, f, 0:S0],
                                     start=(f == 0), stop=(f == FO - 1))
                nc.scalar.copy(ysl0[:, 0, :], p2[:, 0:S0])
                # Tile fails to encode the trigger's deferred RAW dep on
                # the eviction as a hardware wait (CoreSim enforces it
                # structurally, real HW races).  A Pool-engine read of
                # the evicted tile parks the in-order Pool sequencer on
                # a properly-encoded wait; the nosync edge pins the
                # trigger behind it.
                g0 = nc.gpsimd.tensor_scalar_add(
                    guard[:], ysl0[:, 0, 0:16], 0.0)
                t0 = nc.gpsimd.trigger_dma(count=None, queue_num=0)
                _nsdep(t0, g0)
                p2b = psA.tile([P, TA], f32, name="p2s1", tag="pA")
                for f in range(FO):
                    nc.tensor.matmul(p2b[:, 0:S1],
                                     w2sb[:, KO - 1, f * P:(f + 1) * P],
                                     hA[:, f, S0:TA],
                                     start=(f == 0), stop=(f == FO - 1))
                nc.vector.tensor_scalar_add(ysl1[:, 0, :], p2b[:, 0:S1], 0.0)
                g1 = nc.gpsimd.tensor_scalar_add(
                    guard[:], ysl1[:, 0, 0:16], 0.0)
                t1 = nc.gpsimd.trigger_dma(count=None, queue_num=1)
                _nsdep(t1, g1)
                # No explicit wait on sem0/sem1: Tile's teardown drain
                # already waits for the prep DMA-completion sems (and the
                # scheduler would hoist a bare wait_ge above the triggers,
                # deadlocking the Pool queue).
            else:
                # fallback: column-split HWDGE stores
                subs = [TA - TA // 4, TA // 4] if TA >= 256 else [TA]
                c0 = 0
                for s, sub in enumerate(subs):
                    ysb = ypool.tile([P, sub], bf16, tag="yA3", name="yA3")
                    gemm2A(KO - 1, c0, c0 + sub, ysb,
                           use_act=(s % 2 == 0),
                           dma_eng=nc.scalar if s < len(subs) - 1 else None)
                    c0 += sub

    nc.compile()
    if use_sw:
        _mirror_inc_swdge_updates(nc)
    _CACHE[key] = (nc, use_sw)
    return _CACHE[key]


def _mirror_inc_swdge_updates(nc):
    """Expose InstIncSwdgeSem's payload-encoded semaphore bumps as
    sync_info updates.

    Tile's teardown reconciles the SWDGE DMA-lane semaphores with
    InstIncSwdgeSem bumps whose sems live in the instruction payload,
    not in sync_info.  The timeline cost model only sees sync_info, so
    without this mirror the final barrier waits on the lane sems and
    the simulation deadlocks.  The duplicate update is harmless for
    execution: the waits are >= and the teardown range-clears the sems.
    """
    import concourse.mybir as mybir
    from concourse import bass_isa

    for blk in nc.m.functions[0].blocks:
        for ins in blk.instructions:
            if not isinstance(ins, bass_isa.InstIncSwdgeSem):
                continue
            if ins._mode != "add":
                continue
            ups = list(ins.sync_info.on_update) if ins.sync_info else []
            for i, (val, nm) in enumerate(
                    zip(ins._sem_values, ins._sem_names)):
                if val:
                    ups.append(mybir.SyncUpdate(
                        sync_type="semaphore", id=ins._sem_id_base + i,
                        update_mode="sem-add-imm", update_value=val,
                        ant_name=nm))
            waits = list(ins.sync_info.on_wait) if ins.sync_info else []
            ins.sync_info = mybir.SyncInfo(on_wait=waits, on_update=ups)


_last = {}


def _pack_inputs(xs, w_htoh4, w_h4toh, idx_split, C, KO, FO, use_sw):
    bf16 = ml_dtypes.bfloat16
    chunks = _chunks_for(C)
    TA = chunks[0]
    TB = chunks[1] if len(chunks) > 1 else 0
    nfo1 = min(FO1, FO)
    RS = TA + nfo1 * P
    d_model = KO * P
    # idx i lives at [i % 16, i // 16]; the 16-partition wrap must be
    # replicated across all partition groups — the Q7 core serving SWDGE
    # queue k reads a channel stripe that depends on k.
    sidx_h = np.empty((P, 8), dtype=np.int16)
    for p in range(P):
        for s in range(8):
            sidx_h[p, s] = s * 16 + (p % 16)
    in_maps = []
    for e in range(NUM_EXPERT):
        idx = idx_split[e]
        cnt = len(idx)
        xT = np.zeros((d_model, C), dtype=np.float32)
        if cnt:
            xT[:, :cnt] = xs[idx].T
        xk = xT.reshape(KO, P, C)                          # [ko, p, c]
        w1t = w_htoh4[e].T.reshape(KO, P, FO, P)          # [ko, p, fo, f]
        rows = []
        for ko in range(KO):
            rows.append(xk[ko, :, :TA])                   # x-ko  (P, TA)
            rows.append(w1t[ko, :, :nfo1, :].reshape(P, nfo1 * P))
        xw_h = np.concatenate(rows, axis=1)               # (P, KO*RS)
        if TB:
            xB = xk[:, :, TA:C].transpose(1, 0, 2).reshape(P, KO * TB)
            xw_h = np.concatenate([xw_h, xB], axis=1)
        w1b_h = w1t[:, :, nfo1:, :].transpose(1, 2, 0, 3) \
            .reshape(P, FO - nfo1, KO * P)
        w2t = w_h4toh[e].T.reshape(FO, P, KO, P)          # [fo, p, do, d]
        w2_h = w2t.transpose(1, 2, 0, 3).reshape(P, KO, FO * P)
        m = {
            "xw": np.ascontiguousarray(xw_h.astype(bf16)),
            "w1b": np.ascontiguousarray(w1b_h.astype(bf16)),
            "w2": np.ascontiguousarray(w2_h.astype(bf16)),
        }
        if use_sw:
            m["sidx"] = sidx_h
        in_maps.append(m)
    return in_maps


def kernel(inp, gate_idx, gate_score, w_htoh4, w_h4toh):
    inp = np.ascontiguousarray(np.asarray(inp, dtype=np.float32))
    gate_idx = np.asarray(gate_idx)
    gate_score = np.asarray(gate_score, dtype=np.float32)
    w_htoh4 = np.asarray(w_htoh4, dtype=np.float32)
    w_h4toh = np.asarray(w_h4toh, dtype=np.float32)

    B, d_model = inp.shape
    n_expert, d_ff, _ = w_htoh4.shape
    assert n_expert == NUM_EXPERT
    KO = d_model // P
    FO = d_ff // P

    gi = gate_idx.astype(np.int64)
    order = np.argsort(gi, kind="stable")
    counts = np.bincount(gi, minlength=NUM_EXPERT)
    idx_split = np.split(order, np.cumsum(counts)[:-1])

    C = max(int(-(-counts.max() // 16) * 16), 256)
    TA = _chunks_for(C)[0]

    scores_flat = gate_score.reshape(-1)
    xs = inp * scores_flat[:, None]

    nc, use_sw = _build(C, KO, FO)
    in_maps = _pack_inputs(xs, w_htoh4, w_h4toh, idx_split, C, KO, FO,
                           use_sw)

    from concourse import bass_utils
    res = bass_utils.run_bass_kernel_spmd(nc, in_maps,
                                          core_ids=list(range(N_CORES)))

    _last.update(nc=nc, in_maps=in_maps, res=res, C=C, KO=KO, FO=FO)

    y_full = np.empty((B, d_model), dtype=np.float32)
    for e in range(NUM_EXPERT):
        idx = idx_split[e]
        if len(idx) == 0:
            continue
        yt_h = res.results[e]["yt"].astype(np.float32)  # (P, KO*C)
        yA = yt_h[:, :KO * TA].reshape(P, KO, TA)
        if use_sw:
            yA = np.concatenate(
                [yA[:, :KO - 1, :],
                 res.results[e]["ylast"].astype(np.float32)[:, None, :]],
                axis=1)
        if C > TA:
            yB = yt_h[:, KO * TA:].reshape(P, KO, C - TA)
            yk = np.concatenate([yA, yB], axis=2)
        else:
            yk = yA
        yT = yk.transpose(1, 0, 2).reshape(d_model, C)
        y_full[idx] = yT[:, :len(idx)].T
    out = y_full[0::2] + y_full[1::2]
    return np.ascontiguousarray(out, dtype=np.float32)


# revision 16
# speedup vs baseline: 1.0286x; 1.0141x over previous
"""MoE (BruteForceMoELinear) Trainium2 kernel — bf16 expert-parallel.

Strategy: expert-parallel across 8 NeuronCores.  The host dispatches
token rows by `gate_idx` (stable sort), folds the per-row gate score
into the activations (scores >= 0 commute through ReLU), pads each
expert's batch to capacity C, and hands core e bf16-packed operands.

Per-core compute: y_e^T = W2_e @ relu(W1_e @ x_e^T), bf16 matmuls with
fp32 PSUM accumulation.  Tokens split into a big chunk A (<=512 cols)
and a small remainder B.  GEMM1 opens ko-major over the first FO1
f-groups so the PE consumes each (W1-ko, x-ko) row-DMA the moment it
lands; W1-ko and x-ko are packed into a single DRAM row per ko to
minimize per-DMA descriptor-generation serialization.  The rest runs
fo-major against streamed W1, with B's tiny groups woven between A
groups.

Output tail: a plain HWDGE store pays ~625ns descriptor-gen + ~650ns
DGE->DMA latency + ~900ns completion-semaphore propagation after the
final eviction, so the last d-group is drained through SWDGE instead —
dma_scatter_add descriptors are pre-generated on the Pool engine
mid-kernel (prepare_only) into per-piece queues, and each piece fires
with a cheap trigger_dma the moment its PSUM eviction lands (dest
pre-zeroed, so add == store).  The small B remainder and the earlier
d-groups finish long before the end and keep the normal HWDGE path.
"""

import numpy as np
import ml_dtypes

import os

NUM_EXPERT = 8
N_CORES = 8
P = 128
FO1 = int(os.environ.get("K_FO1", "6"))  # ko-major head fo-groups
_CUT = int(os.environ.get("K_CUT", "2"))     # W1 cols in first DMA piece
_S1 = int(os.environ.get("K_S1", "128"))     # final scatter piece cols

_CACHE = {}


def _chunks_for(C):
    if C <= 512:
        return [C]
    assert C <= 1024
    return [512, C - 512]


def _build(C, KO, FO, repeat=1):
    key = (C, KO, FO, repeat)
    if key in _CACHE:
        return _CACHE[key]

    import concourse.mybir as mybir
    import concourse.tile as tile
    from concourse import bacc
    from concourse.instruction_name_ordered_set import InstructionNameOrderedSet

    def _nsdep(inst, dep):
        s = InstructionNameOrderedSet()
        s.add(dep.ins.name)
        inst.ins.add_nosync_dependencies_from(s)

    f32 = mybir.dt.float32
    bf16 = mybir.dt.bfloat16
    i16 = mybir.dt.int16
    chunks = _chunks_for(C)
    TA = chunks[0]
    TB = chunks[1] if len(chunks) > 1 else 0
    nfo1 = min(FO1, FO)
    FOB = FO - nfo1
    RS = TA + nfo1 * P           # row stride: x-ko | w1a-ko
    XWN = KO * RS + KO * TB      # + xB appended at the end
    use_sw = (TA % 128 == 0) and TA >= 2 * _S1  # SWDGE tail path
    S1 = _S1
    S0 = TA - S1

    nc = bacc.Bacc("TRN2", target_bir_lowering=False, debug=False,
                   num_devices=N_CORES,
                   num_swdge_queues=2 if use_sw else 1)

    xw = nc.dram_tensor("xw", (P, XWN), bf16, kind="ExternalInput")
    w1b = nc.dram_tensor("w1b", (P, FOB, KO * P), bf16, kind="ExternalInput")
    w2 = nc.dram_tensor("w2", (P, KO, FO * P), bf16, kind="ExternalInput")
    yt = nc.dram_tensor("yt", (P, KO * C), bf16, kind="ExternalOutput")
    if use_sw:
        sidx = nc.dram_tensor("sidx", (P, 8), i16, kind="ExternalInput")
        ylast = nc.dram_tensor("ylast", (P, TA), f32, kind="ExternalOutput")

    with tile.TileContext(nc) as tc:
        with tc.tile_pool(name="wpool", bufs=1) as wpool, \
             tc.tile_pool(name="ypool", bufs=4) as ypool, \
             tc.tile_pool(name="psA", bufs=6, space="PSUM") as psA, \
             tc.tile_pool(name="psB", bufs=2, space="PSUM") as psB:

            xwsb = wpool.tile([P, XWN], bf16, name="xwsb")
            w1bsb = (wpool.tile([P, FOB, KO * P], bf16, name="w1bsb")
                     if FOB else None)
            w2sb = wpool.tile([P, KO, FO * P], bf16, name="w2sb")
            hA = wpool.tile([P, FO, TA], bf16, name="hA")
            hB = wpool.tile([P, FO, TB], bf16, name="hB") if TB else None
            if use_sw:
                sidxsb = wpool.tile([P, 8], i16, name="sidxsb")
                ysl0 = wpool.tile([P, 1, S0], f32, name="ysl0")
                ysl1 = wpool.tile([P, 1, S1], f32, name="ysl1")
                guard = wpool.tile([P, 16], f32, name="guard")

            def xA_ap(ko):
                return xwsb[:, ko * RS:ko * RS + TA]

            def xB_ap(ko):
                return xwsb[:, KO * RS + ko * TB:KO * RS + (ko + 1) * TB]

            def w1_ap(f, ko):
                if f < nfo1:
                    off = ko * RS + TA + f * P
                    return xwsb[:, off:off + P]
                return w1bsb[:, f - nfo1, ko * P:(ko + 1) * P]

            # --- DMAs: emission order == consumption order -------------
            cut = TA + _CUT * P if nfo1 >= _CUT else RS
            nc.sync.dma_start(xwsb[:, 0:cut], xw.ap()[:, 0:cut])
            if cut < RS:
                nc.sync.dma_start(xwsb[:, cut:RS], xw.ap()[:, cut:RS])
            for ko in range(1, KO):
                hi = (ko + 1) * RS if ko < KO - 1 else XWN
                nc.sync.dma_start(xwsb[:, ko * RS:hi], xw.ap()[:, ko * RS:hi])
            fo = 0
            while fo < FOB:
                hi = min(fo + int(os.environ.get("K_W1B", "4")), FOB)
                nc.sync.dma_start(w1bsb[:, fo:hi, :], w1b.ap()[:, fo:hi, :])
                fo = hi
            if use_sw:
                nc.sync.dma_start(sidxsb[:], sidx.ap()[:])
            nc.sync.dma_start(w2sb[:, 0:2, :], w2.ap()[:, 0:2, :])
            nc.sync.dma_start(w2sb[:, 2:KO, :], w2.ap()[:, 2:KO, :])
            if use_sw:
                # No explicit zeroing of ylast: both execution paths
                # (native run_neff and the PJRT redirect) pre-zero
                # ExternalOutput buffers, so scatter-add == store.
                sem0 = nc.alloc_semaphore("sdma0")
                sem1 = nc.alloc_semaphore("sdma1")
                prep0 = nc.gpsimd.dma_scatter_add(
                    ylast.ap()[:, 0:S0], ysl0[:], sidxsb[:], P, P, S0,
                    elem_step=TA, prepare_only=True, sem=sem0, queue_num=0,
                    single_packet=False)
                prep1 = nc.gpsimd.dma_scatter_add(
                    ylast.ap()[:, S0:TA], ysl1[:], sidxsb[:], P, P, S1,
                    elem_step=TA, prepare_only=True, sem=sem1, queue_num=1,
                    single_packet=False)

            def evict1(dst, src, use_act):
                if use_act:
                    nc.scalar.activation(dst, src,
                                         mybir.ActivationFunctionType.Relu)
                else:
                    nc.vector.tensor_scalar_max(dst, src, 0.0)

            # Keep-warm bridge: the cost model resets the PE p-state
            # anchor when the PE idles more than ~0.8us, and the ramp to
            # full rate takes 3us from the anchor.  Emit a chain of tiny
            # matmuls, each gated by a ~0.6us Pool-engine memset, so PE
            # activity recurs every <0.7us until the first real matmul
            # (~3.5us, after the row-0 DMA) — which then runs full-rate.
            warm = wpool.tile([P, 16], bf16, name="warm")
            NPACE = 18
            pace = wpool.tile([P, (NPACE + 1) * 16], bf16, name="pace")
            bconst = nc.const_aps.aps[(mybir.dt.bfloat16, 1.0)]
            nc.tensor.ldweights(bconst)
            nc.vector.memset(warm[:], 0.0)
            nc.vector.memset(pace[:, 0:16], 0.0)
            wps = psB.tile([P, 16], f32, name="wps", tag="pB")
            nc.tensor.matmul(wps[0:16, :], warm[:], warm[:],
                             start=True, stop=True)
            for k in range(NPACE):
                nc.vector.tensor_scalar_add(
                    pace[:, (k + 1) * 16:(k + 2) * 16],
                    pace[:, k * 16:(k + 1) * 16], 0.0)
                nc.tensor.matmul(
                    wps[0:16, :], warm[:],
                    pace[:, (k + 1) * 16:(k + 2) * 16],
                    start=True, stop=True)

            # --- GEMM1 phase 1: ko-major over fo 0..nfo1 on chunk A ----
            p1s = [psA.tile([P, TA], f32, name=f"p1f{f}", tag="pA")
                   for f in range(nfo1)]
            for ko in range(KO):
                for f in range(nfo1):
                    nc.tensor.matmul(p1s[f][:], w1_ap(f, ko), xA_ap(ko),
                                     start=(ko == 0), stop=(ko == KO - 1))
                    if ko == KO - 1:
                        evict1(hA[:, f, :], p1s[f][:], f % 2 == 0)

            # --- GEMM1 phase 2: fo-major, B's groups interleaved -------
            def gemm1B(f):
                pb = psB.tile([P, TB], f32, name="pb", tag="pB")
                for ko in range(KO):
                    nc.tensor.matmul(pb[:], w1_ap(f, ko), xB_ap(ko),
                                     start=(ko == 0), stop=(ko == KO - 1))
                nc.vector.tensor_scalar_max(hB[:, f, :], pb[:], 0.0)

            bq = list(range(FO)) if TB else []
            NB = len(bq)
            nA2 = max(FO - nfo1, 1)
            for i, f in enumerate(range(nfo1, FO)):
                p1 = psA.tile([P, TA], f32, name="p1", tag="pA")
                for ko in range(KO):
                    nc.tensor.matmul(p1[:], w1_ap(f, ko), xA_ap(ko),
                                     start=(ko == 0), stop=(ko == KO - 1))
                evict1(hA[:, f, :], p1[:], True)
                ntake = ((i + 1) * NB) // nA2 - (i * NB) // nA2
                for _ in range(ntake):
                    gemm1B(bq.pop(0))
            for f in bq:
                gemm1B(f)

            # --- GEMM2 --------------------------------------------------
            def gemm2A(do, c0, c1, ysb, use_act=True, dma_eng=None):
                p2 = psA.tile([P, TA], f32, name="p2", tag="pA")
                for f in range(FO):
                    nc.tensor.matmul(p2[:, 0:c1 - c0],
                                     w2sb[:, do, f * P:(f + 1) * P],
                                     hA[:, f, c0:c1],
                                     start=(f == 0), stop=(f == FO - 1))
                if use_act:
                    nc.scalar.copy(ysb[:], p2[:, 0:c1 - c0])
                else:
                    nc.vector.tensor_scalar_add(ysb[:], p2[:, 0:c1 - c0], 0.0)
                (dma_eng or nc.sync).dma_start(
                    yt.ap()[:, do * TA + c0:do * TA + c1], ysb[:])

            def gemm2B(do, ysbB):
                pb = psB.tile([P, TB], f32, name="p2b", tag="pB")
                for f in range(FO):
                    nc.tensor.matmul(pb[:],
                                     w2sb[:, do, f * P:(f + 1) * P],
                                     hB[:, f, :],
                                     start=(f == 0), stop=(f == FO - 1))
                nc.vector.tensor_scalar_add(
                    ysbB[:, do * TB:(do + 1) * TB], pb[:], 0.0)

            # B first: its tokens finish (and store) long before the
            # tail so its HWDGE latency chain never gates the kernel end.
            if TB:
                ysbB = ypool.tile([P, KO * TB], bf16, tag="yB", name="yB")
                for do in range(KO):
                    gemm2B(do, ysbB)
                nc.sync.dma_start(yt.ap()[:, KO * TA:KO * C], ysbB[:])
            for do in range(KO - 1):
                ysb = ypool.tile([P, TA], bf16, tag="yA", name="yA")
                gemm2A(do, 0, TA, ysb)

            if use_sw:
                # Last d-group via pre-generated SWDGE descriptors: the
                # trigger skips HWDGE desc-gen and the DGE->DMA handoff,
                # so the final store starts right after its eviction.
                p2 = psA.tile([P, TA], f32, name="p2s0", tag="pA")
                for f in range(FO):
                    nc.tensor.matmul(p2[:, 0:S0],
                                     w2sb[:, KO - 1, f * P:(f + 1) * P],
                                     hA[:, f, 0:S0],
                                     start=(f == 0), stop=(f == FO - 1))
                e0 = nc.scalar.copy(ysl0[:, 0, :], p2[:, 0:S0])
                # Tile fails to encode the trigger's deferred RAW dep on
                # the eviction as a hardware wait (CoreSim enforces it
                # structurally, real HW races).  A Pool-engine read of
                # the evicted tile parks the in-order Pool sequencer on
                # a properly-encoded wait; the nosync edge pins the
                # trigger behind it.
                g0 = nc.gpsimd.tensor_scalar_add(
                    guard[:], ysl0[:, 0, 0:16], 0.0)
                t0 = nc.gpsimd.trigger_dma(count=None, queue_num=0)
                _nsdep(t0, g0)
                # Unpin the prep from the eviction's stream position so
                # its ~1us Pool desc-gen runs early, not between the
                # evict and the trigger.  Safe: desc-gen only reads idxs;
                # the data read happens at trigger time, and the guard
                # enforces evict -> trigger on hardware.
                prep0.ins.try_remove_dependency(e0.ins.name)
                p2b = psA.tile([P, TA], f32, name="p2s1", tag="pA")
                for f in range(FO):
                    nc.tensor.matmul(p2b[:, 0:S1],
                                     w2sb[:, KO - 1, f * P:(f + 1) * P],
                                     hA[:, f, S0:TA],
                                     start=(f == 0), stop=(f == FO - 1))
                e1 = nc.vector.tensor_scalar_add(ysl1[:, 0, :], p2b[:, 0:S1],
                                                 0.0)
                g1 = nc.gpsimd.tensor_scalar_add(
                    guard[:], ysl1[:, 0, 0:16], 0.0)
                t1 = nc.gpsimd.trigger_dma(count=None, queue_num=1)
                _nsdep(t1, g1)
                prep1.ins.try_remove_dependency(e1.ins.name)
                # No explicit wait on sem0/sem1: Tile's teardown drain
                # already waits for the prep DMA-completion sems (and the
                # scheduler would hoist a bare wait_ge above the triggers,
                # deadlocking the Pool queue).
            else:
                # fallback: column-split HWDGE stores
                subs = [TA - TA // 4, TA // 4] if TA >= 256 else [TA]
                c0 = 0
                for s, sub in enumerate(subs):
                    ysb = ypool.tile([P, sub], bf16, tag="yA3", name="yA3")
                    gemm2A(KO - 1, c0, c0 + sub, ysb,
                           use_act=(s % 2 == 0),
                           dma_eng=nc.scalar if s < len(subs) - 1 else None)
                    c0 += sub

    nc.compile()
    if use_sw:
        _mirror_inc_swdge_updates(nc)
    _CACHE[key] = (nc, use_sw)
    return _CACHE[key]


def _mirror_inc_swdge_updates(nc):
    """Expose InstIncSwdgeSem's payload-encoded semaphore bumps as
    sync_info updates.

    Tile's teardown reconciles the SWDGE DMA-lane semaphores with
    InstIncSwdgeSem bumps whose sems live in the instruction payload,
    not in sync_info.  The timeline cost model only sees sync_info, so
    without this mirror the final barrier waits on the lane sems and
    the simulation deadlocks.  The duplicate update is harmless for
    execution: the waits are >= and the teardown range-clears the sems.
    """
    import concourse.mybir as mybir
    from concourse import bass_isa

    for blk in nc.m.functions[0].blocks:
        for ins in blk.instructions:
            if not isinstance(ins, bass_isa.InstIncSwdgeSem):
                continue
            if ins._mode != "add":
                continue
            ups = list(ins.sync_info.on_update) if ins.sync_info else []
            for i, (val, nm) in enumerate(
                    zip(ins._sem_values, ins._sem_names)):
                if val:
                    ups.append(mybir.SyncUpdate(
                        sync_type="semaphore", id=ins._sem_id_base + i,
                        update_mode="sem-add-imm", update_value=val,
                        ant_name=nm))
            waits = list(ins.sync_info.on_wait) if ins.sync_info else []
            ins.sync_info = mybir.SyncInfo(on_wait=waits, on_update=ups)


_last = {}


def _pack_inputs(xs, w_htoh4, w_h4toh, idx_split, C, KO, FO, use_sw):
    bf16 = ml_dtypes.bfloat16
    chunks = _chunks_for(C)
    TA = chunks[0]
    TB = chunks[1] if len(chunks) > 1 else 0
    nfo1 = min(FO1, FO)
    RS = TA + nfo1 * P
    d_model = KO * P
    # idx i lives at [i % 16, i // 16]; the 16-partition wrap must be
    # replicated across all partition groups — the Q7 core serving SWDGE
    # queue k reads a channel stripe that depends on k.
    sidx_h = np.empty((P, 8), dtype=np.int16)
    for p in range(P):
        for s in range(8):
            sidx_h[p, s] = s * 16 + (p % 16)
    in_maps = []
    for e in range(NUM_EXPERT):
        idx = idx_split[e]
        cnt = len(idx)
        xT = np.zeros((d_model, C), dtype=np.float32)
        if cnt:
            xT[:, :cnt] = xs[idx].T
        xk = xT.reshape(KO, P, C)                          # [ko, p, c]
        w1t = w_htoh4[e].T.reshape(KO, P, FO, P)          # [ko, p, fo, f]
        rows = []
        for ko in range(KO):
            rows.append(xk[ko, :, :TA])                   # x-ko  (P, TA)
            rows.append(w1t[ko, :, :nfo1, :].reshape(P, nfo1 * P))
        xw_h = np.concatenate(rows, axis=1)               # (P, KO*RS)
        if TB:
            xB = xk[:, :, TA:C].transpose(1, 0, 2).reshape(P, KO * TB)
            xw_h = np.concatenate([xw_h, xB], axis=1)
        w1b_h = w1t[:, :, nfo1:, :].transpose(1, 2, 0, 3) \
            .reshape(P, FO - nfo1, KO * P)
        w2t = w_h4toh[e].T.reshape(FO, P, KO, P)          # [fo, p, do, d]
        w2_h = w2t.transpose(1, 2, 0, 3).reshape(P, KO, FO * P)
        m = {
            "xw": np.ascontiguousarray(xw_h.astype(bf16)),
            "w1b": np.ascontiguousarray(w1b_h.astype(bf16)),
            "w2": np.ascontiguousarray(w2_h.astype(bf16)),
        }
        if use_sw:
            m["sidx"] = sidx_h
        in_maps.append(m)
    return in_maps


def kernel(inp, gate_idx, gate_score, w_htoh4, w_h4toh):
    inp = np.ascontiguousarray(np.asarray(inp, dtype=np.float32))
    gate_idx = np.asarray(gate_idx)
    gate_score = np.asarray(gate_score, dtype=np.float32)
    w_htoh4 = np.asarray(w_htoh4, dtype=np.float32)
    w_h4toh = np.asarray(w_h4toh, dtype=np.float32)

    B, d_model = inp.shape
    n_expert, d_ff, _ = w_htoh4.shape
    assert n_expert == NUM_EXPERT
    KO = d_model // P
    FO = d_ff // P

    gi = gate_idx.astype(np.int64)
    order = np.argsort(gi, kind="stable")
    counts = np.bincount(gi, minlength=NUM_EXPERT)
    idx_split = np.split(order, np.cumsum(counts)[:-1])

    C = max(int(-(-counts.max() // 16) * 16), 256)
    TA = _chunks_for(C)[0]

    scores_flat = gate_score.reshape(-1)
    xs = inp * scores_flat[:, None]

    nc, use_sw = _build(C, KO, FO)
    in_maps = _pack_inputs(xs, w_htoh4, w_h4toh, idx_split, C, KO, FO,
                           use_sw)

    from concourse import bass_utils
    res = bass_utils.run_bass_kernel_spmd(nc, in_maps,
                                          core_ids=list(range(N_CORES)))

    _last.update(nc=nc, in_maps=in_maps, res=res, C=C, KO=KO, FO=FO)

    y_full = np.empty((B, d_model), dtype=np.float32)
    for e in range(NUM_EXPERT):
        idx = idx_split[e]
        if len(idx) == 0:
            continue
        yt_h = res.results[e]["yt"].astype(np.float32)  # (P, KO*C)
        yA = yt_h[:, :KO * TA].reshape(P, KO, TA)
        if use_sw:
            yA = np.concatenate(
                [yA[:, :KO - 1, :],
                 res.results[e]["ylast"].astype(np.float32)[:, None, :]],
                axis=1)
        if C > TA:
            yB = yt_h[:, KO * TA:].reshape(P, KO, C - TA)
            yk = np.concatenate([yA, yB], axis=2)
        else:
            yk = yA
        yT = yk.transpose(1, 0, 2).reshape(d_model, C)
        y_full[idx] = yT[:, :len(idx)].T
    out = y_full[0::2] + y_full[1::2]
    return np.ascontiguousarray(out, dtype=np.float32)


# revision 17
# speedup vs baseline: 1.0325x; 1.0038x over previous
"""MoE (BruteForceMoELinear) Trainium2 kernel — bf16 expert-parallel.

Strategy: expert-parallel across 8 NeuronCores.  The host dispatches
token rows by `gate_idx` (stable sort), folds the per-row gate score
into the activations (scores >= 0 commute through ReLU), pads each
expert's batch to capacity C, and hands core e bf16-packed operands.

Per-core compute: y_e^T = W2_e @ relu(W1_e @ x_e^T), bf16 matmuls with
fp32 PSUM accumulation.  Tokens split into a big chunk A (<=512 cols)
and a small remainder B.  GEMM1 opens ko-major over the first FO1
f-groups so the PE consumes each (W1-ko, x-ko) row-DMA the moment it
lands; W1-ko and x-ko are packed into a single DRAM row per ko to
minimize per-DMA descriptor-generation serialization.  The rest runs
fo-major against streamed W1, with B's tiny groups woven between A
groups.

Output tail: a plain HWDGE store pays ~625ns descriptor-gen + ~650ns
DGE->DMA latency + ~900ns completion-semaphore propagation after the
final eviction, so the last d-group is drained through SWDGE instead —
dma_scatter_add descriptors are pre-generated on the Pool engine
mid-kernel (prepare_only) into per-piece queues, and each piece fires
with a cheap trigger_dma the moment its PSUM eviction lands (dest
pre-zeroed, so add == store).  The small B remainder and the earlier
d-groups finish long before the end and keep the normal HWDGE path.
"""

import numpy as np
import ml_dtypes

import os

NUM_EXPERT = 8
N_CORES = 8
P = 128
FO1 = int(os.environ.get("K_FO1", "6"))  # ko-major head fo-groups
_CUT = int(os.environ.get("K_CUT", "2"))     # W1 cols in first DMA piece
_S1 = int(os.environ.get("K_S1", "128"))     # final scatter piece cols

_CACHE = {}


def _chunks_for(C):
    if C <= 512:
        return [C]
    assert C <= 1024
    return [512, C - 512]


def _build(C, KO, FO, repeat=1):
    key = (C, KO, FO, repeat)
    if key in _CACHE:
        return _CACHE[key]

    import concourse.mybir as mybir
    import concourse.tile as tile
    from concourse import bacc
    from concourse.instruction_name_ordered_set import InstructionNameOrderedSet

    def _nsdep(inst, dep):
        s = InstructionNameOrderedSet()
        s.add(dep.ins.name)
        inst.ins.add_nosync_dependencies_from(s)

    f32 = mybir.dt.float32
    bf16 = mybir.dt.bfloat16
    i16 = mybir.dt.int16
    chunks = _chunks_for(C)
    TA = chunks[0]
    TB = chunks[1] if len(chunks) > 1 else 0
    nfo1 = min(FO1, FO)
    FOB = FO - nfo1
    RS = TA + nfo1 * P           # row stride: x-ko | w1a-ko
    XWN = KO * RS + KO * TB      # + xB appended at the end
    use_sw = (TA % 128 == 0) and TA >= 2 * _S1  # SWDGE tail path
    S1 = _S1
    S0 = TA - S1

    nc = bacc.Bacc("TRN2", target_bir_lowering=False, debug=False,
                   num_devices=N_CORES,
                   num_swdge_queues=2 if use_sw else 1)

    xw = nc.dram_tensor("xw", (P, XWN), bf16, kind="ExternalInput")
    w1b = nc.dram_tensor("w1b", (P, FOB, KO * P), bf16, kind="ExternalInput")
    w2 = nc.dram_tensor("w2", (P, KO, FO * P), bf16, kind="ExternalInput")
    yt = nc.dram_tensor("yt", (P, KO * C), bf16, kind="ExternalOutput")
    if use_sw:
        sidx = nc.dram_tensor("sidx", (P, 8), i16, kind="ExternalInput")
        ylast = nc.dram_tensor("ylast", (P, TA), f32, kind="ExternalOutput")

    with tile.TileContext(nc) as tc:
        with tc.tile_pool(name="wpool", bufs=1) as wpool, \
             tc.tile_pool(name="ypool", bufs=4) as ypool, \
             tc.tile_pool(name="psA", bufs=6, space="PSUM") as psA, \
             tc.tile_pool(name="psB", bufs=2, space="PSUM") as psB:

            xwsb = wpool.tile([P, XWN], bf16, name="xwsb")
            w1bsb = (wpool.tile([P, FOB, KO * P], bf16, name="w1bsb")
                     if FOB else None)
            w2sb = wpool.tile([P, KO, FO * P], bf16, name="w2sb")
            hA = wpool.tile([P, FO, TA], bf16, name="hA")
            hB = wpool.tile([P, FO, TB], bf16, name="hB") if TB else None
            if use_sw:
                sidxsb = wpool.tile([P, 8], i16, name="sidxsb")
                ysl0 = wpool.tile([P, 1, S0], f32, name="ysl0")
                ysl1 = wpool.tile([P, 1, S1], f32, name="ysl1")
                guard = wpool.tile([P, 16], f32, name="guard")

            def xA_ap(ko):
                return xwsb[:, ko * RS:ko * RS + TA]

            def xB_ap(ko):
                return xwsb[:, KO * RS + ko * TB:KO * RS + (ko + 1) * TB]

            def w1_ap(f, ko):
                if f < nfo1:
                    off = ko * RS + TA + f * P
                    return xwsb[:, off:off + P]
                return w1bsb[:, f - nfo1, ko * P:(ko + 1) * P]

            # --- DMAs: emission order == consumption order -------------
            cut = TA + _CUT * P if nfo1 >= _CUT else RS
            nc.sync.dma_start(xwsb[:, 0:cut], xw.ap()[:, 0:cut])
            if cut < RS:
                nc.sync.dma_start(xwsb[:, cut:RS], xw.ap()[:, cut:RS])
            for ko in range(1, KO):
                hi = (ko + 1) * RS if ko < KO - 1 else XWN
                nc.sync.dma_start(xwsb[:, ko * RS:hi], xw.ap()[:, ko * RS:hi])
            fo = 0
            while fo < FOB:
                hi = min(fo + int(os.environ.get("K_W1B", "4")), FOB)
                nc.sync.dma_start(w1bsb[:, fo:hi, :], w1b.ap()[:, fo:hi, :])
                fo = hi
            if use_sw:
                nc.sync.dma_start(sidxsb[:], sidx.ap()[:])
            nc.sync.dma_start(w2sb[:, 0:2, :], w2.ap()[:, 0:2, :])
            nc.sync.dma_start(w2sb[:, 2:KO, :], w2.ap()[:, 2:KO, :])
            if use_sw:
                # No explicit zeroing of ylast: both execution paths
                # (native run_neff and the PJRT redirect) pre-zero
                # ExternalOutput buffers, so scatter-add == store.
                sem0 = nc.alloc_semaphore("sdma0")
                sem1 = nc.alloc_semaphore("sdma1")
                prep0 = nc.gpsimd.dma_scatter_add(
                    ylast.ap()[:, 0:S0], ysl0[:], sidxsb[:], P, P, S0,
                    elem_step=TA, prepare_only=True, sem=sem0, queue_num=0,
                    single_packet=False)
                prep1 = nc.gpsimd.dma_scatter_add(
                    ylast.ap()[:, S0:TA], ysl1[:], sidxsb[:], P, P, S1,
                    elem_step=TA, prepare_only=True, sem=sem1, queue_num=1,
                    single_packet=False)

            def evict1(dst, src, use_act):
                if use_act:
                    nc.scalar.activation(dst, src,
                                         mybir.ActivationFunctionType.Relu)
                else:
                    nc.vector.tensor_scalar_max(dst, src, 0.0)

            # Keep-warm bridge: the cost model resets the PE p-state
            # anchor when the PE idles more than ~0.8us, and the ramp to
            # full rate takes 3us from the anchor.  Emit a chain of tiny
            # matmuls, each gated by a ~0.6us Pool-engine memset, so PE
            # activity recurs every <0.7us until the first real matmul
            # (~3.5us, after the row-0 DMA) — which then runs full-rate.
            warm = wpool.tile([P, 16], bf16, name="warm")
            NPACE = 18
            pace = wpool.tile([P, (NPACE + 1) * 16], bf16, name="pace")
            bconst = nc.const_aps.aps[(mybir.dt.bfloat16, 1.0)]
            nc.tensor.ldweights(bconst)
            nc.vector.memset(warm[:], 0.0)
            nc.vector.memset(pace[:, 0:16], 0.0)
            wps = psB.tile([P, 16], f32, name="wps", tag="pB")
            nc.tensor.matmul(wps[0:16, :], warm[:], warm[:],
                             start=True, stop=True)
            for k in range(NPACE):
                nc.vector.tensor_scalar_add(
                    pace[:, (k + 1) * 16:(k + 2) * 16],
                    pace[:, k * 16:(k + 1) * 16], 0.0)
                nc.tensor.matmul(
                    wps[0:16, :], warm[:],
                    pace[:, (k + 1) * 16:(k + 2) * 16],
                    start=True, stop=True)

            # --- GEMM1 phase 1: ko-major over fo 0..nfo1 on chunk A ----
            p1s = [psA.tile([P, TA], f32, name=f"p1f{f}", tag="pA")
                   for f in range(nfo1)]
            for ko in range(KO):
                for f in range(nfo1):
                    nc.tensor.matmul(p1s[f][:], w1_ap(f, ko), xA_ap(ko),
                                     start=(ko == 0), stop=(ko == KO - 1))
                    if ko == KO - 1:
                        evict1(hA[:, f, :], p1s[f][:], f % 2 == 0)

            # --- GEMM1 phase 2: fo-major, B's groups interleaved -------
            def gemm1B(f):
                pb = psB.tile([P, TB], f32, name="pb", tag="pB")
                for ko in range(KO):
                    nc.tensor.matmul(pb[:], w1_ap(f, ko), xB_ap(ko),
                                     start=(ko == 0), stop=(ko == KO - 1))
                nc.vector.tensor_scalar_max(hB[:, f, :], pb[:], 0.0)

            bq = list(range(FO)) if TB else []
            NB = len(bq)
            nA2 = max(FO - nfo1, 1)
            for i, f in enumerate(range(nfo1, FO)):
                p1 = psA.tile([P, TA], f32, name="p1", tag="pA")
                for ko in range(KO):
                    nc.tensor.matmul(p1[:], w1_ap(f, ko), xA_ap(ko),
                                     start=(ko == 0), stop=(ko == KO - 1))
                evict1(hA[:, f, :], p1[:], True)
                ntake = ((i + 1) * NB) // nA2 - (i * NB) // nA2
                for _ in range(ntake):
                    gemm1B(bq.pop(0))
            for f in bq:
                gemm1B(f)

            # --- GEMM2 --------------------------------------------------
            def gemm2A(do, c0, c1, ysb, use_act=True, dma_eng=None):
                p2 = psA.tile([P, TA], f32, name="p2", tag="pA")
                for f in range(FO):
                    nc.tensor.matmul(p2[:, 0:c1 - c0],
                                     w2sb[:, do, f * P:(f + 1) * P],
                                     hA[:, f, c0:c1],
                                     start=(f == 0), stop=(f == FO - 1))
                if use_act:
                    nc.scalar.copy(ysb[:], p2[:, 0:c1 - c0])
                else:
                    nc.vector.tensor_scalar_add(ysb[:], p2[:, 0:c1 - c0], 0.0)
                (dma_eng or nc.sync).dma_start(
                    yt.ap()[:, do * TA + c0:do * TA + c1], ysb[:])

            def gemm2B(do, ysbB):
                pb = psB.tile([P, TB], f32, name="p2b", tag="pB")
                for f in range(FO):
                    nc.tensor.matmul(pb[:],
                                     w2sb[:, do, f * P:(f + 1) * P],
                                     hB[:, f, :],
                                     start=(f == 0), stop=(f == FO - 1))
                nc.vector.tensor_scalar_add(
                    ysbB[:, do * TB:(do + 1) * TB], pb[:], 0.0)

            # B's tiny groups woven between the A d-groups (hides their
            # PE.SEQ decode behind long A matmuls); all B work and its
            # store finish during do=2, well before the scatter tail.
            ysbB = (ypool.tile([P, KO * TB], bf16, tag="yB", name="yB")
                    if TB else None)
            for do in range(KO - 1):
                ysb = ypool.tile([P, TA], bf16, tag="yA", name="yA")
                gemm2A(do, 0, TA, ysb)
                if TB:
                    gemm2B(do, ysbB)
                    if do == KO - 2:
                        gemm2B(KO - 1, ysbB)
                        nc.sync.dma_start(yt.ap()[:, KO * TA:KO * C],
                                          ysbB[:])

            if use_sw:
                # Last d-group via pre-generated SWDGE descriptors: the
                # trigger skips HWDGE desc-gen and the DGE->DMA handoff,
                # so the final store starts right after its eviction.
                p2 = psA.tile([P, TA], f32, name="p2s0", tag="pA")
                for f in range(FO):
                    nc.tensor.matmul(p2[:, 0:S0],
                                     w2sb[:, KO - 1, f * P:(f + 1) * P],
                                     hA[:, f, 0:S0],
                                     start=(f == 0), stop=(f == FO - 1))
                e0 = nc.scalar.copy(ysl0[:, 0, :], p2[:, 0:S0])
                # Tile fails to encode the trigger's deferred RAW dep on
                # the eviction as a hardware wait (CoreSim enforces it
                # structurally, real HW races).  A Pool-engine read of
                # the evicted tile parks the in-order Pool sequencer on
                # a properly-encoded wait; the nosync edge pins the
                # trigger behind it.
                g0 = nc.gpsimd.tensor_scalar_add(
                    guard[:], ysl0[:, 0, 0:16], 0.0)
                t0 = nc.gpsimd.trigger_dma(count=None, queue_num=0)
                _nsdep(t0, g0)
                # Unpin the prep from the eviction's stream position so
                # its ~1us Pool desc-gen runs early, not between the
                # evict and the trigger.  Safe: desc-gen only reads idxs;
                # the data read happens at trigger time, and the guard
                # enforces evict -> trigger on hardware.
                prep0.ins.try_remove_dependency(e0.ins.name)
                p2b = psA.tile([P, TA], f32, name="p2s1", tag="pA")
                for f in range(FO):
                    nc.tensor.matmul(p2b[:, 0:S1],
                                     w2sb[:, KO - 1, f * P:(f + 1) * P],
                                     hA[:, f, S0:TA],
                                     start=(f == 0), stop=(f == FO - 1))
                e1 = nc.vector.tensor_scalar_add(ysl1[:, 0, :], p2b[:, 0:S1],
                                                 0.0)
                g1 = nc.gpsimd.tensor_scalar_add(
                    guard[:], ysl1[:, 0, 0:16], 0.0)
                t1 = nc.gpsimd.trigger_dma(count=None, queue_num=1)
                _nsdep(t1, g1)
                prep1.ins.try_remove_dependency(e1.ins.name)
                # No explicit wait on sem0/sem1: Tile's teardown drain
                # already waits for the prep DMA-completion sems (and the
                # scheduler would hoist a bare wait_ge above the triggers,
                # deadlocking the Pool queue).
            else:
                # fallback: column-split HWDGE stores
                subs = [TA - TA // 4, TA // 4] if TA >= 256 else [TA]
                c0 = 0
                for s, sub in enumerate(subs):
                    ysb = ypool.tile([P, sub], bf16, tag="yA3", name="yA3")
                    gemm2A(KO - 1, c0, c0 + sub, ysb,
                           use_act=(s % 2 == 0),
                           dma_eng=nc.scalar if s < len(subs) - 1 else None)
                    c0 += sub

    nc.compile()
    if use_sw:
        _mirror_inc_swdge_updates(nc)
    _CACHE[key] = (nc, use_sw)
    return _CACHE[key]


def _mirror_inc_swdge_updates(nc):
    """Expose InstIncSwdgeSem's payload-encoded semaphore bumps as
    sync_info updates.

    Tile's teardown reconciles the SWDGE DMA-lane semaphores with
    InstIncSwdgeSem bumps whose sems live in the instruction payload,
    not in sync_info.  The timeline cost model only sees sync_info, so
    without this mirror the final barrier waits on the lane sems and
    the simulation deadlocks.  The duplicate update is harmless for
    execution: the waits are >= and the teardown range-clears the sems.
    """
    import concourse.mybir as mybir
    from concourse import bass_isa

    for blk in nc.m.functions[0].blocks:
        for ins in blk.instructions:
            if not isinstance(ins, bass_isa.InstIncSwdgeSem):
                continue
            if ins._mode != "add":
                continue
            ups = list(ins.sync_info.on_update) if ins.sync_info else []
            for i, (val, nm) in enumerate(
                    zip(ins._sem_values, ins._sem_names)):
                if val:
                    ups.append(mybir.SyncUpdate(
                        sync_type="semaphore", id=ins._sem_id_base + i,
                        update_mode="sem-add-imm", update_value=val,
                        ant_name=nm))
            waits = list(ins.sync_info.on_wait) if ins.sync_info else []
            ins.sync_info = mybir.SyncInfo(on_wait=waits, on_update=ups)


_last = {}


def _pack_inputs(xs, w_htoh4, w_h4toh, idx_split, C, KO, FO, use_sw):
    bf16 = ml_dtypes.bfloat16
    chunks = _chunks_for(C)
    TA = chunks[0]
    TB = chunks[1] if len(chunks) > 1 else 0
    nfo1 = min(FO1, FO)
    RS = TA + nfo1 * P
    d_model = KO * P
    # idx i lives at [i % 16, i // 16]; the 16-partition wrap must be
    # replicated across all partition groups — the Q7 core serving SWDGE
    # queue k reads a channel stripe that depends on k.
    sidx_h = np.empty((P, 8), dtype=np.int16)
    for p in range(P):
        for s in range(8):
            sidx_h[p, s] = s * 16 + (p % 16)
    in_maps = []
    for e in range(NUM_EXPERT):
        idx = idx_split[e]
        cnt = len(idx)
        xT = np.zeros((d_model, C), dtype=np.float32)
        if cnt:
            xT[:, :cnt] = xs[idx].T
        xk = xT.reshape(KO, P, C)                          # [ko, p, c]
        w1t = w_htoh4[e].T.reshape(KO, P, FO, P)          # [ko, p, fo, f]
        rows = []
        for ko in range(KO):
            rows.append(xk[ko, :, :TA])                   # x-ko  (P, TA)
            rows.append(w1t[ko, :, :nfo1, :].reshape(P, nfo1 * P))
        xw_h = np.concatenate(rows, axis=1)               # (P, KO*RS)
        if TB:
            xB = xk[:, :, TA:C].transpose(1, 0, 2).reshape(P, KO * TB)
            xw_h = np.concatenate([xw_h, xB], axis=1)
        w1b_h = w1t[:, :, nfo1:, :].transpose(1, 2, 0, 3) \
            .reshape(P, FO - nfo1, KO * P)
        w2t = w_h4toh[e].T.reshape(FO, P, KO, P)          # [fo, p, do, d]
        w2_h = w2t.transpose(1, 2, 0, 3).reshape(P, KO, FO * P)
        m = {
            "xw": np.ascontiguousarray(xw_h.astype(bf16)),
            "w1b": np.ascontiguousarray(w1b_h.astype(bf16)),
            "w2": np.ascontiguousarray(w2_h.astype(bf16)),
        }
        if use_sw:
            m["sidx"] = sidx_h
        in_maps.append(m)
    return in_maps


def kernel(inp, gate_idx, gate_score, w_htoh4, w_h4toh):
    inp = np.ascontiguousarray(np.asarray(inp, dtype=np.float32))
    gate_idx = np.asarray(gate_idx)
    gate_score = np.asarray(gate_score, dtype=np.float32)
    w_htoh4 = np.asarray(w_htoh4, dtype=np.float32)
    w_h4toh = np.asarray(w_h4toh, dtype=np.float32)

    B, d_model = inp.shape
    n_expert, d_ff, _ = w_htoh4.shape
    assert n_expert == NUM_EXPERT
    KO = d_model // P
    FO = d_ff // P

    gi = gate_idx.astype(np.int64)
    order = np.argsort(gi, kind="stable")
    counts = np.bincount(gi, minlength=NUM_EXPERT)
    idx_split = np.split(order, np.cumsum(counts)[:-1])

    C = max(int(-(-counts.max() // 16) * 16), 256)
    TA = _chunks_for(C)[0]

    scores_flat = gate_score.reshape(-1)
    xs = inp * scores_flat[:, None]

    nc, use_sw = _build(C, KO, FO)
    in_maps = _pack_inputs(xs, w_htoh4, w_h4toh, idx_split, C, KO, FO,
                           use_sw)

    from concourse import bass_utils
    res = bass_utils.run_bass_kernel_spmd(nc, in_maps,
                                          core_ids=list(range(N_CORES)))

    _last.update(nc=nc, in_maps=in_maps, res=res, C=C, KO=KO, FO=FO)

    y_full = np.empty((B, d_model), dtype=np.float32)
    for e in range(NUM_EXPERT):
        idx = idx_split[e]
        if len(idx) == 0:
            continue
        yt_h = res.results[e]["yt"].astype(np.float32)  # (P, KO*C)
        yA = yt_h[:, :KO * TA].reshape(P, KO, TA)
        if use_sw:
            yA = np.concatenate(
                [yA[:, :KO - 1, :],
                 res.results[e]["ylast"].astype(np.float32)[:, None, :]],
                axis=1)
        if C > TA:
            yB = yt_h[:, KO * TA:].reshape(P, KO, C - TA)
            yk = np.concatenate([yA, yB], axis=2)
        else:
            yk = yA
        yT = yk.transpose(1, 0, 2).reshape(d_model, C)
        y_full[idx] = yT[:, :len(idx)].T
    out = y_full[0::2] + y_full[1::2]
    return np.ascontiguousarray(out, dtype=np.float32)


# revision 22
# speedup vs baseline: 1.0332x; 1.0007x over previous
"""MoE (BruteForceMoELinear) Trainium2 kernel — bf16 expert-parallel.

Strategy: expert-parallel across 8 NeuronCores.  The host dispatches
token rows by `gate_idx` (stable sort), folds the per-row gate score
into the activations (scores >= 0 commute through ReLU), pads each
expert's batch to capacity C, and hands core e bf16-packed operands.

Per-core compute: y_e^T = W2_e @ relu(W1_e @ x_e^T), bf16 matmuls with
fp32 PSUM accumulation.  Tokens split into a big chunk A (<=512 cols)
and a small remainder B.  GEMM1 opens ko-major over the first FO1
f-groups so the PE consumes each (W1-ko, x-ko) row-DMA the moment it
lands; W1-ko and x-ko are packed into a single DRAM row per ko to
minimize per-DMA descriptor-generation serialization.  The rest runs
fo-major against streamed W1, with B's tiny groups woven between A
groups.

Output tail: a plain HWDGE store pays ~625ns descriptor-gen + ~650ns
DGE->DMA latency + ~900ns completion-semaphore propagation after the
final eviction, so the last d-group is drained through SWDGE instead —
dma_scatter_add descriptors are pre-generated on the Pool engine
mid-kernel (prepare_only) into per-piece queues, and each piece fires
with a cheap trigger_dma the moment its PSUM eviction lands (dest
pre-zeroed, so add == store).  The small B remainder and the earlier
d-groups finish long before the end and keep the normal HWDGE path.
"""

import numpy as np
import ml_dtypes

import os

NUM_EXPERT = 8
N_CORES = 8
P = 128
FO1 = int(os.environ.get("K_FO1", "6"))  # ko-major head fo-groups
_CUT = int(os.environ.get("K_CUT", "2"))     # W1 cols in first DMA piece
_S1 = int(os.environ.get("K_S1", "128"))     # final scatter piece cols

_CACHE = {}


def _chunks_for(C):
    if C <= 512:
        return [C]
    assert C <= 1024
    return [512, C - 512]


def _build(C, KO, FO, repeat=1):
    key = (C, KO, FO, repeat)
    if key in _CACHE:
        return _CACHE[key]

    import concourse.mybir as mybir
    import concourse.tile as tile
    from concourse import bacc
    from concourse.instruction_name_ordered_set import InstructionNameOrderedSet

    def _nsdep(inst, dep):
        s = InstructionNameOrderedSet()
        s.add(dep.ins.name)
        inst.ins.add_nosync_dependencies_from(s)

    f32 = mybir.dt.float32
    bf16 = mybir.dt.bfloat16
    i16 = mybir.dt.int16
    chunks = _chunks_for(C)
    TA = chunks[0]
    TB = chunks[1] if len(chunks) > 1 else 0
    nfo1 = min(FO1, FO)
    FOB = FO - nfo1
    RS = TA + nfo1 * P           # row stride: x-ko | w1a-ko
    XWN = KO * RS + KO * TB      # + xB appended at the end
    use_sw = (TA % 128 == 0) and TA >= 2 * _S1  # SWDGE tail path
    S1 = _S1            # tail cols split off the last d-group
    S0 = TA - S1
    S1A = S1 // 2       # penultimate piece (queue 1)
    S1B = S1 - S1A      # final piece (queue 2) — smallest evict+store

    nc = bacc.Bacc("TRN2", target_bir_lowering=False, debug=False,
                   num_devices=N_CORES,
                   num_swdge_queues=3 if use_sw else 1)

    xw = nc.dram_tensor("xw", (P, XWN), bf16, kind="ExternalInput")
    w1b = nc.dram_tensor("w1b", (P, FOB, KO * P), bf16, kind="ExternalInput")
    w2 = nc.dram_tensor("w2", (P, KO, FO * P), bf16, kind="ExternalInput")
    yt = nc.dram_tensor("yt", (P, KO * C), bf16, kind="ExternalOutput")
    if use_sw:
        sidx = nc.dram_tensor("sidx", (P, 8), i16, kind="ExternalInput")
        ylast = nc.dram_tensor("ylast", (P, TA), f32, kind="ExternalOutput")

    with tile.TileContext(nc) as tc:
        with tc.tile_pool(name="wpool", bufs=1) as wpool, \
             tc.tile_pool(name="ypool", bufs=4) as ypool, \
             tc.tile_pool(name="psA", bufs=6, space="PSUM") as psA, \
             tc.tile_pool(name="psB", bufs=2, space="PSUM") as psB:

            xwsb = wpool.tile([P, XWN], bf16, name="xwsb")
            w1bsb = (wpool.tile([P, FOB, KO * P], bf16, name="w1bsb")
                     if FOB else None)
            w2sb = wpool.tile([P, KO, FO * P], bf16, name="w2sb")
            hA = wpool.tile([P, FO, TA], bf16, name="hA")
            hB = wpool.tile([P, FO, TB], bf16, name="hB") if TB else None
            if use_sw:
                sidxsb = wpool.tile([P, 8], i16, name="sidxsb")
                ysl0 = wpool.tile([P, 1, S0], f32, name="ysl0")
                ysl1a = wpool.tile([P, 1, S1A], f32, name="ysl1a")
                ysl1b = wpool.tile([P, 1, S1B], f32, name="ysl1b")
                guard = wpool.tile([P, 16], f32, name="guard")

            def xA_ap(ko):
                return xwsb[:, ko * RS:ko * RS + TA]

            def xB_ap(ko):
                return xwsb[:, KO * RS + ko * TB:KO * RS + (ko + 1) * TB]

            def w1_ap(f, ko):
                if f < nfo1:
                    off = ko * RS + TA + f * P
                    return xwsb[:, off:off + P]
                return w1bsb[:, f - nfo1, ko * P:(ko + 1) * P]

            # --- DMAs: emission order == consumption order -------------
            cut = TA + _CUT * P if nfo1 >= _CUT else RS
            nc.sync.dma_start(xwsb[:, 0:cut], xw.ap()[:, 0:cut])
            if cut < RS:
                nc.sync.dma_start(xwsb[:, cut:RS], xw.ap()[:, cut:RS])
            for ko in range(1, KO):
                hi = (ko + 1) * RS if ko < KO - 1 else XWN
                nc.sync.dma_start(xwsb[:, ko * RS:hi], xw.ap()[:, ko * RS:hi])
            fo = 0
            while fo < FOB:
                hi = min(fo + int(os.environ.get("K_W1B", "4")), FOB)
                nc.sync.dma_start(w1bsb[:, fo:hi, :], w1b.ap()[:, fo:hi, :])
                fo = hi
            if use_sw:
                nc.sync.dma_start(sidxsb[:], sidx.ap()[:])
            nc.sync.dma_start(w2sb[:, 0:2, :], w2.ap()[:, 0:2, :])
            nc.sync.dma_start(w2sb[:, 2:KO, :], w2.ap()[:, 2:KO, :])
            if use_sw:
                # No explicit zeroing of ylast: both execution paths
                # (native run_neff and the PJRT redirect) pre-zero
                # ExternalOutput buffers, so scatter-add == store.
                sem0 = nc.alloc_semaphore("sdma0")
                sem1 = nc.alloc_semaphore("sdma1")
                sem2 = nc.alloc_semaphore("sdma2")
                prep0 = nc.gpsimd.dma_scatter_add(
                    ylast.ap()[:, 0:S0], ysl0[:], sidxsb[:], P, P, S0,
                    elem_step=TA, prepare_only=True, sem=sem0, queue_num=0,
                    single_packet=False)
                prep1 = nc.gpsimd.dma_scatter_add(
                    ylast.ap()[:, S0:S0 + S1A], ysl1a[:], sidxsb[:], P, P,
                    S1A, elem_step=TA, prepare_only=True, sem=sem1,
                    queue_num=1, single_packet=False)
                prep2 = nc.gpsimd.dma_scatter_add(
                    ylast.ap()[:, S0 + S1A:TA], ysl1b[:], sidxsb[:], P, P,
                    S1B, elem_step=TA, prepare_only=True, sem=sem2,
                    queue_num=2, single_packet=False)

            def evict1(dst, src, use_act):
                if use_act:
                    nc.scalar.activation(dst, src,
                                         mybir.ActivationFunctionType.Relu)
                else:
                    nc.vector.tensor_scalar_max(dst, src, 0.0)

            # Keep-warm bridge: the cost model resets the PE p-state
            # anchor when the PE idles more than ~0.8us, and the ramp to
            # full rate takes 3us from the anchor.  Emit a chain of tiny
            # matmuls, each gated by a ~0.6us Pool-engine memset, so PE
            # activity recurs every <0.7us until the first real matmul
            # (~3.5us, after the row-0 DMA) — which then runs full-rate.
            warm = wpool.tile([P, 16], bf16, name="warm")
            NPACE = 18
            pace = wpool.tile([P, (NPACE + 1) * 16], bf16, name="pace")
            bconst = nc.const_aps.aps[(mybir.dt.bfloat16, 1.0)]
            nc.tensor.ldweights(bconst)
            nc.vector.memset(warm[:], 0.0)
            nc.vector.memset(pace[:, 0:16], 0.0)
            wps = psB.tile([P, 16], f32, name="wps", tag="pB")
            nc.tensor.matmul(wps[0:16, :], warm[:], warm[:],
                             start=True, stop=True)
            for k in range(NPACE):
                nc.vector.tensor_scalar_add(
                    pace[:, (k + 1) * 16:(k + 2) * 16],
                    pace[:, k * 16:(k + 1) * 16], 0.0)
                nc.tensor.matmul(
                    wps[0:16, :], warm[:],
                    pace[:, (k + 1) * 16:(k + 2) * 16],
                    start=True, stop=True)

            # --- GEMM1 phase 1: ko-major over fo 0..nfo1 on chunk A ----
            p1s = [psA.tile([P, TA], f32, name=f"p1f{f}", tag="pA")
                   for f in range(nfo1)]
            for ko in range(KO):
                for f in range(nfo1):
                    nc.tensor.matmul(p1s[f][:], w1_ap(f, ko), xA_ap(ko),
                                     start=(ko == 0), stop=(ko == KO - 1))
                    if ko == KO - 1:
                        evict1(hA[:, f, :], p1s[f][:], f % 2 == 0)

            # --- GEMM1 phase 2: fo-major, B's groups interleaved -------
            def gemm1B(f):
                pb = psB.tile([P, TB], f32, name="pb", tag="pB")
                for ko in range(KO):
                    nc.tensor.matmul(pb[:], w1_ap(f, ko), xB_ap(ko),
                                     start=(ko == 0), stop=(ko == KO - 1))
                nc.vector.tensor_scalar_max(hB[:, f, :], pb[:], 0.0)

            bq = list(range(FO)) if TB else []
            NB = len(bq)
            nA2 = max(FO - nfo1, 1)
            for i, f in enumerate(range(nfo1, FO)):
                p1 = psA.tile([P, TA], f32, name="p1", tag="pA")
                for ko in range(KO):
                    nc.tensor.matmul(p1[:], w1_ap(f, ko), xA_ap(ko),
                                     start=(ko == 0), stop=(ko == KO - 1))
                evict1(hA[:, f, :], p1[:], True)
                ntake = ((i + 1) * NB) // nA2 - (i * NB) // nA2
                for _ in range(ntake):
                    gemm1B(bq.pop(0))
            for f in bq:
                gemm1B(f)

            # --- GEMM2 --------------------------------------------------
            def gemm2A(do, c0, c1, ysb, use_act=True, dma_eng=None):
                p2 = psA.tile([P, TA], f32, name="p2", tag="pA")
                for f in range(FO):
                    nc.tensor.matmul(p2[:, 0:c1 - c0],
                                     w2sb[:, do, f * P:(f + 1) * P],
                                     hA[:, f, c0:c1],
                                     start=(f == 0), stop=(f == FO - 1))
                if use_act:
                    nc.scalar.copy(ysb[:], p2[:, 0:c1 - c0])
                else:
                    nc.vector.tensor_scalar_add(ysb[:], p2[:, 0:c1 - c0], 0.0)
                (dma_eng or nc.sync).dma_start(
                    yt.ap()[:, do * TA + c0:do * TA + c1], ysb[:])

            def gemm2B(do, ysbB):
                pb = psB.tile([P, TB], f32, name="p2b", tag="pB")
                for f in range(FO):
                    nc.tensor.matmul(pb[:],
                                     w2sb[:, do, f * P:(f + 1) * P],
                                     hB[:, f, :],
                                     start=(f == 0), stop=(f == FO - 1))
                nc.vector.tensor_scalar_add(
                    ysbB[:, do * TB:(do + 1) * TB], pb[:], 0.0)

            # B's tiny groups woven between the A d-groups (hides their
            # PE.SEQ decode behind long A matmuls); all B work and its
            # store finish during do=2, well before the scatter tail.
            ysbB = (ypool.tile([P, KO * TB], bf16, tag="yB", name="yB")
                    if TB else None)
            for do in range(KO - 1):
                ysb = ypool.tile([P, TA], bf16, tag="yA", name="yA")
                gemm2A(do, 0, TA, ysb)
                if TB:
                    gemm2B(do, ysbB)
                    if do == KO - 2:
                        gemm2B(KO - 1, ysbB)
                        nc.sync.dma_start(yt.ap()[:, KO * TA:KO * C],
                                          ysbB[:])

            if use_sw:
                # Last d-group via pre-generated SWDGE descriptors: the
                # trigger skips HWDGE desc-gen and the DGE->DMA handoff,
                # so the final store starts right after its eviction.
                p2 = psA.tile([P, TA], f32, name="p2s0", tag="pA")
                for f in range(FO):
                    nc.tensor.matmul(p2[:, 0:S0],
                                     w2sb[:, KO - 1, f * P:(f + 1) * P],
                                     hA[:, f, 0:S0],
                                     start=(f == 0), stop=(f == FO - 1))
                e0 = nc.scalar.copy(ysl0[:, 0, :], p2[:, 0:S0])
                # Tile fails to encode the trigger's deferred RAW dep on
                # the eviction as a hardware wait (CoreSim enforces it
                # structurally, real HW races).  A Pool-engine read of
                # the evicted tile parks the in-order Pool sequencer on
                # a properly-encoded wait; the nosync edge pins the
                # trigger behind it.
                g0 = nc.gpsimd.tensor_scalar_add(
                    guard[:], ysl0[:, 0, 0:16], 0.0)
                t0 = nc.gpsimd.trigger_dma(count=None, queue_num=0)
                _nsdep(t0, g0)
                # Unpin the prep from the eviction's stream position so
                # its ~1us Pool desc-gen runs early, not between the
                # evict and the trigger.  Safe: desc-gen only reads idxs;
                # the data read happens at trigger time, and the guard
                # enforces evict -> trigger on hardware.
                prep0.ins.try_remove_dependency(e0.ins.name)
                p2b = psA.tile([P, TA], f32, name="p2s1", tag="pA")
                for f in range(FO):
                    nc.tensor.matmul(p2b[:, 0:S1A],
                                     w2sb[:, KO - 1, f * P:(f + 1) * P],
                                     hA[:, f, S0:S0 + S1A],
                                     start=(f == 0), stop=(f == FO - 1))
                e1 = nc.vector.tensor_scalar_add(ysl1a[:, 0, :],
                                                 p2b[:, 0:S1A], 0.0)
                g1 = nc.gpsimd.tensor_scalar_add(
                    guard[:], ysl1a[:, 0, 0:16], 0.0)
                t1 = nc.gpsimd.trigger_dma(count=None, queue_num=1)
                _nsdep(t1, g1)
                prep1.ins.try_remove_dependency(e1.ins.name)
                p2c = psA.tile([P, TA], f32, name="p2s2", tag="pA")
                for f in range(FO):
                    nc.tensor.matmul(p2c[:, 0:S1B],
                                     w2sb[:, KO - 1, f * P:(f + 1) * P],
                                     hA[:, f, S0 + S1A:TA],
                                     start=(f == 0), stop=(f == FO - 1))
                e2 = nc.vector.tensor_scalar_add(ysl1b[:, 0, :],
                                                 p2c[:, 0:S1B], 0.0)
                g2 = nc.gpsimd.tensor_scalar_add(
                    guard[:], ysl1b[:, 0, 0:16], 0.0)
                t2 = nc.gpsimd.trigger_dma(count=None, queue_num=2)
                _nsdep(t2, g2)
                prep2.ins.try_remove_dependency(e2.ins.name)
                # No explicit wait on sem0/sem1: Tile's teardown drain
                # already waits for the prep DMA-completion sems (and the
                # scheduler would hoist a bare wait_ge above the triggers,
                # deadlocking the Pool queue).
            else:
                # fallback: column-split HWDGE stores
                subs = [TA - TA // 4, TA // 4] if TA >= 256 else [TA]
                c0 = 0
                for s, sub in enumerate(subs):
                    ysb = ypool.tile([P, sub], bf16, tag="yA3", name="yA3")
                    gemm2A(KO - 1, c0, c0 + sub, ysb,
                           use_act=(s % 2 == 0),
                           dma_eng=nc.scalar if s < len(subs) - 1 else None)
                    c0 += sub

    nc.compile()
    if use_sw:
        _mirror_inc_swdge_updates(nc)
    _CACHE[key] = (nc, use_sw)
    return _CACHE[key]


def _mirror_inc_swdge_updates(nc):
    """Expose InstIncSwdgeSem's payload-encoded semaphore bumps as
    sync_info updates.

    Tile's teardown reconciles the SWDGE DMA-lane semaphores with
    InstIncSwdgeSem bumps whose sems live in the instruction payload,
    not in sync_info.  The timeline cost model only sees sync_info, so
    without this mirror the final barrier waits on the lane sems and
    the simulation deadlocks.  The duplicate update is harmless for
    execution: the waits are >= and the teardown range-clears the sems.
    """
    import concourse.mybir as mybir
    from concourse import bass_isa

    for blk in nc.m.functions[0].blocks:
        for ins in blk.instructions:
            if not isinstance(ins, bass_isa.InstIncSwdgeSem):
                continue
            if ins._mode != "add":
                continue
            ups = list(ins.sync_info.on_update) if ins.sync_info else []
            for i, (val, nm) in enumerate(
                    zip(ins._sem_values, ins._sem_names)):
                if val:
                    ups.append(mybir.SyncUpdate(
                        sync_type="semaphore", id=ins._sem_id_base + i,
                        update_mode="sem-add-imm", update_value=val,
                        ant_name=nm))
            waits = list(ins.sync_info.on_wait) if ins.sync_info else []
            ins.sync_info = mybir.SyncInfo(on_wait=waits, on_update=ups)


_last = {}


def _pack_inputs(xs, w_htoh4, w_h4toh, idx_split, C, KO, FO, use_sw):
    bf16 = ml_dtypes.bfloat16
    chunks = _chunks_for(C)
    TA = chunks[0]
    TB = chunks[1] if len(chunks) > 1 else 0
    nfo1 = min(FO1, FO)
    RS = TA + nfo1 * P
    d_model = KO * P
    # idx i lives at [i % 16, i // 16]; the 16-partition wrap must be
    # replicated across all partition groups — the Q7 core serving SWDGE
    # queue k reads a channel stripe that depends on k.
    sidx_h = np.empty((P, 8), dtype=np.int16)
    for p in range(P):
        for s in range(8):
            sidx_h[p, s] = s * 16 + (p % 16)
    in_maps = []
    for e in range(NUM_EXPERT):
        idx = idx_split[e]
        cnt = len(idx)
        xT = np.zeros((d_model, C), dtype=np.float32)
        if cnt:
            xT[:, :cnt] = xs[idx].T
        xk = xT.reshape(KO, P, C)                          # [ko, p, c]
        w1t = w_htoh4[e].T.reshape(KO, P, FO, P)          # [ko, p, fo, f]
        rows = []
        for ko in range(KO):
            rows.append(xk[ko, :, :TA])                   # x-ko  (P, TA)
            rows.append(w1t[ko, :, :nfo1, :].reshape(P, nfo1 * P))
        xw_h = np.concatenate(rows, axis=1)               # (P, KO*RS)
        if TB:
            xB = xk[:, :, TA:C].transpose(1, 0, 2).reshape(P, KO * TB)
            xw_h = np.concatenate([xw_h, xB], axis=1)
        w1b_h = w1t[:, :, nfo1:, :].transpose(1, 2, 0, 3) \
            .reshape(P, FO - nfo1, KO * P)
        w2t = w_h4toh[e].T.reshape(FO, P, KO, P)          # [fo, p, do, d]
        w2_h = w2t.transpose(1, 2, 0, 3).reshape(P, KO, FO * P)
        m = {
            "xw": np.ascontiguousarray(xw_h.astype(bf16)),
            "w1b": np.ascontiguousarray(w1b_h.astype(bf16)),
            "w2": np.ascontiguousarray(w2_h.astype(bf16)),
        }
        if use_sw:
            m["sidx"] = sidx_h
        in_maps.append(m)
    return in_maps


def kernel(inp, gate_idx, gate_score, w_htoh4, w_h4toh):
    inp = np.ascontiguousarray(np.asarray(inp, dtype=np.float32))
    gate_idx = np.asarray(gate_idx)
    gate_score = np.asarray(gate_score, dtype=np.float32)
    w_htoh4 = np.asarray(w_htoh4, dtype=np.float32)
    w_h4toh = np.asarray(w_h4toh, dtype=np.float32)

    B, d_model = inp.shape
    n_expert, d_ff, _ = w_htoh4.shape
    assert n_expert == NUM_EXPERT
    KO = d_model // P
    FO = d_ff // P

    gi = gate_idx.astype(np.int64)
    order = np.argsort(gi, kind="stable")
    counts = np.bincount(gi, minlength=NUM_EXPERT)
    idx_split = np.split(order, np.cumsum(counts)[:-1])

    C = max(int(-(-counts.max() // 16) * 16), 256)
    TA = _chunks_for(C)[0]

    scores_flat = gate_score.reshape(-1)
    xs = inp * scores_flat[:, None]

    nc, use_sw = _build(C, KO, FO)
    in_maps = _pack_inputs(xs, w_htoh4, w_h4toh, idx_split, C, KO, FO,
                           use_sw)

    from concourse import bass_utils
    res = bass_utils.run_bass_kernel_spmd(nc, in_maps,
                                          core_ids=list(range(N_CORES)))

    _last.update(nc=nc, in_maps=in_maps, res=res, C=C, KO=KO, FO=FO)

    y_full = np.empty((B, d_model), dtype=np.float32)
    for e in range(NUM_EXPERT):
        idx = idx_split[e]
        if len(idx) == 0:
            continue
        yt_h = res.results[e]["yt"].astype(np.float32)  # (P, KO*C)
        yA = yt_h[:, :KO * TA].reshape(P, KO, TA)
        if use_sw:
            yA = np.concatenate(
                [yA[:, :KO - 1, :],
                 res.results[e]["ylast"].astype(np.float32)[:, None, :]],
                axis=1)
        if C > TA:
            yB = yt_h[:, KO * TA:].reshape(P, KO, C - TA)
            yk = np.concatenate([yA, yB], axis=2)
        else:
            yk = yA
        yT = yk.transpose(1, 0, 2).reshape(d_model, C)
        y_full[idx] = yT[:, :len(idx)].T
    out = y_full[0::2] + y_full[1::2]
    return np.ascontiguousarray(out, dtype=np.float32)
